# revision 53
# baseline (speedup 1.0000x reference)
"""ATKT (LSTM + degenerate causal attention + FC) Trainium2 kernel, v3.

Full inputs in, full outputs out. Shards batch (64) across 8 NeuronCores
(8 sequences per core), runs a Bass/Tile kernel per core, reassembles.

The Tile scheduler's cost model charges ~100ns (SEM_DELAY) per data-
dependency edge, so the LSTM recurrence (512 serial steps) is bound by
the number of serial edges per step, not engine throughput. v3 cuts the
baseline's 8-edge step to 6 edges:

  Whh-matmuls -> tanh(all 8 gate chunks, one Act) -> STT1 -> STT2
      -> {t1 = (tau_o+1)*s', q = s'^2 on Pool} -> tq = t1*q (Pool)

and feeds the PE *two* rhs streams instead of h:
  gates = xg + Whh @ h~' = xg - 12*Whh @ t1 + Whh @ (t1*q)
(the cubic-tanh h~' = (q-12)*t1 = -12*h~ is materialized off the
critical cycle for attention/FC only; the -1/12 scale is folded into
all weight consumers on the host).

Other restructurings:
 - xg (the interaction-embedding gate table) is gathered AND transposed
   on the host, so phase 1 is four plain DMAs instead of 32 indirect
   gathers + 32 DMA transposes on engine queues.
 - Attention (64-step granularity) and FC (128-token chunks) are
   emitted as thunks and dripped a few per step into the engine-idle
   windows of the recurrence loop; the last sub-block's pipeline is
   ordered per-sequence (scans then that sequence's FC) to minimize the
   tail.
 - The final sigmoid is computed as tanh on-device (table shared with
   the recurrence) and 0.5*y+0.5 on the host; y is fp16.
"""
import os
import sys
from collections import deque

sys.path.insert(0, "/opt/trn_rl_repo")

import numpy as np
import ml_dtypes

B, T = 64, 512
DC = DR = DL = DA = 256
NC = 1024
N_CORES = 8
BC = B // N_CORES          # sequences per core
TOK = BC * T               # tokens per core (4096)
BLK = 128                  # recurrence block size for phase-3 streaming
NBLK = T // BLK

# Custom DVE ops and gpsimd-STT both fail this container's walrus codegen
# ("ISA wrong length" / "engine check failed") -- stock ops only.
USE_CUSTOM_DVE = False
STT_ON_POOL = False

# ----------------------------------------------------------------------------
# Walrus workaround: this container's neuronxcc rejects >1 sync wait per
# instruction ("Too many sync wait commands"). Split multi-wait instructions
# into single-wait NoOps on the same engine.
# ----------------------------------------------------------------------------


def _apply_tile_patches():
    import bass_rust
    import concourse.tile as tile
    from concourse import mybir

    if getattr(tile.TileContext, "_waitsplit_patched", False):
        return

    _orig_lower = tile.TileContext._lower_ordered_insts

    def _split_waits_in_list(uid, insts, counter):
        new_list = []
        for inst in insts:
            si = inst.sync_info
            if si is not None and len(si.on_wait) > 1:
                waits = list(si.on_wait)
                for w in waits[:-1]:
                    counter[0] += 1
                    nop = mybir.InstNoOp(
                        name=f"waitsplit_{uid}_{counter[0]}",
                        engine=inst.engine,
                        sync_info=bass_rust.SyncInfo(on_wait=[w], on_update=[]),
                        bass_nofuse=True,
                    )
                    new_list.append(nop)
                inst.sync_info = bass_rust.SyncInfo(
                    on_wait=[waits[-1]], on_update=list(si.on_update))
            new_list.append(inst)
        return new_list

    def _patched_lower(self, ordered):
        counter = [0]
        for bb_name in list(ordered.keys()):
            ordered[bb_name] = _split_waits_in_list(self.uid, ordered[bb_name], counter)
        return _orig_lower(self, ordered)

    def _patched_drain_and_barrier(self, tick_clock, wait_clock):
        nc = self.nc
        drain_inst = nc.sync.drain()
        wait_clock.add_sem_waits(
            drain_inst.ins, tile.ScopedClock({None: tick_clock.global_clock}))
        si = drain_inst.ins.sync_info
        if si is not None and len(si.on_wait) > 1:
            waits = list(si.on_wait)
            drain_inst.ins.sync_info = bass_rust.SyncInfo(
                on_wait=waits[:1], on_update=list(si.on_update))
            for w in waits[1:]:
                nop = nc.sync.nop(nofuse=True)
                nop.ins.sync_info = bass_rust.SyncInfo(on_wait=[w], on_update=[])
        nc.all_engine_barrier()
        assert self.sems is not None
        popped = nc._tile_sem_poison_stack.pop()
        assert popped is self._sem_poison
        nc.clear_and_free_semaphores(list(self.sems.allocated().values()))
        nc.all_engine_barrier()

    tile.TileContext._lower_ordered_insts = _patched_lower
    tile.TileContext._drain_and_barrier = _patched_drain_and_barrier
    tile.TileContext._waitsplit_patched = True


# ----------------------------------------------------------------------------
# Custom DVE op: h~ = (tau_o + 1) * s * (1 + C2 * s^2), C2 = -1/12.
# Registered through the documented dve_ops extension point (OPS list);
# the uop program ships in the per-NEFF DVE table, no firmware change.
# ----------------------------------------------------------------------------

_HTILDE = None


def _register_htilde():
    global _HTILDE
    if _HTILDE is not None:
        return _HTILDE
    import concourse.dve_ops as dve_ops
    from concourse.dve_ops import DveOp, OPS, CUSTOM_DVE_SPECS, _SUB_OPCODE_FOR_NAME
    from concourse.dve_spec import Spec, Src0, Src1, C2, One, sq, lower
    from concourse.dve_uop import DveOpSpec, DveVer

    name = "ATKT_HTILDE"
    if name in _SUB_OPCODE_FOR_NAME:
        _HTILDE = next(op for op in OPS if op.name == name)
        return _HTILDE

    body = (Src0 + One) * Src1 * (sq(Src1) * C2 + One)
    spec = Spec(
        body=body,
        reference=lambda in0, in1, s0, s1, imm2: (
            (in0.astype(np.float32) + 1.0)
            * in1.astype(np.float32)
            * (np.square(in1.astype(np.float32)) * imm2 + 1.0)
        ),
    )
    row = max(_SUB_OPCODE_FOR_NAME.values()) + 1
    assert row < 0x20
    _SUB_OPCODE_FOR_NAME[name] = row

    shas = {}
    for ver in ("v3", "v4"):
        uops = lower(spec, ver=ver)
        tmp = DveOpSpec(name=name, opcode=row, uops=uops, rd1_en=True)
        shas[ver] = tmp.sha(ver)
    op = DveOp(name, spec, subdim=False, uops_sha=shas)
    OPS.append(op)
    CUSTOM_DVE_SPECS[name] = spec
    _HTILDE = op
    return op


# ----------------------------------------------------------------------------
# Kernel build
# ----------------------------------------------------------------------------

def build_kernel(t_steps=T, has_mlpb=False, has_fcb=False):
    import concourse.bass as bass
    import concourse.tile as tile
    from concourse import mybir

    _apply_tile_patches()
    htilde_op = _register_htilde() if USE_CUSTOM_DVE else None

    f32 = mybir.dt.float32
    # fp16: |attn_excl| reaches ~320 and fc logits ~43; bf16's 8-bit mantissa
    # costs 4e-2 output error, fp16's 10 bits is fine.
    bf16 = mybir.dt.float16
    i32 = mybir.dt.int32
    AF = mybir.ActivationFunctionType
    OP = mybir.AluOpType

    nc = bass.Bass("TRN2", target_bir_lowering=False, debug=False,
                   num_devices=N_CORES)

    n_tok = BC * t_steps
    n_tc = n_tok // 128            # 128-token chunks
    tc_per_seq = t_steps // 128
    nblk = t_steps // BLK

    # ---- DRAM parameters (per core) ----
    # xg is gathered AND transposed on the host: [blk, p, j, b, t-in-block]
    xg_d = nc.dram_tensor(
        "xg", [t_steps // BLK, 128, 8, BC, BLK], bf16,
        kind="ExternalInput").ap()
    whhT = nc.dram_tensor("whhT", [DL, 2, 4 * DL], bf16, kind="ExternalInput").ap()
    mlpWT = nc.dram_tensor("mlpWT", [DL, DA], bf16, kind="ExternalInput").ap()
    mlpb = nc.dram_tensor("mlpb", [1, DA], bf16, kind="ExternalInput").ap()
    simW = nc.dram_tensor("simW", [DA, 128], bf16, kind="ExternalInput").ap()
    fcWT = nc.dram_tensor("fcWT", [2 * DL, NC], bf16, kind="ExternalInput").ap()
    fcb = nc.dram_tensor("fcb", [1, NC], bf16, kind="ExternalInput").ap()
    yout = nc.dram_tensor("y", [n_tok, NC], bf16, kind="ExternalOutput").ap()

    with tile.TileContext(nc) as tc:
        import contextlib
        with contextlib.ExitStack() as ctx:
            g_pool = ctx.enter_context(tc.tile_pool(name="globals", bufs=1))
            lstm_pool = ctx.enter_context(tc.tile_pool(name="lstm", bufs=1))
            xg_pool = ctx.enter_context(tc.tile_pool(name="xg", bufs=1))
            row_pool = ctx.enter_context(tc.tile_pool(name="rows", bufs=6))
            p3_pool = ctx.enter_context(tc.tile_pool(name="p3", bufs=1))
            p3s_pool = ctx.enter_context(tc.tile_pool(name="p3scratch", bufs=2))
            p3o_pool = ctx.enter_context(tc.tile_pool(name="p3out", bufs=2))
            ps_g = ctx.enter_context(
                tc.tile_pool(name="ps_gates", bufs=2, space="PSUM"))
            ps_att = ctx.enter_context(
                tc.tile_pool(name="ps_att", bufs=1, space="PSUM"))
            ps_out = ctx.enter_context(
                tc.tile_pool(name="ps_out", bufs=4, space="PSUM"))

            # ---- persistent small tiles ----
            ones_f = g_pool.tile([128, 512], f32)
            nc.vector.memset(ones_f, 1.0)
            ones_b = g_pool.tile([128, 512], bf16)
            nc.vector.memset(ones_b, 1.0)
            ident_b = g_pool.tile([128, 128], bf16)
            nc.vector.memset(ident_b, 1.0)
            nc.gpsimd.affine_select(
                out=ident_b, in_=ident_b, pattern=[[-1, 128]],
                compare_op=OP.is_equal, fill=0.0, base=0, channel_multiplier=1)

            whh_sb = g_pool.tile([128, 2, 2, 4 * DL], bf16)
            whh_r = whhT.rearrange("(k p) s g -> p k s g", p=128)
            # split across two queues so neither DMA alone gates step 0
            nc.gpsimd.dma_start(out=whh_sb[:, :, 0, :], in_=whh_r[:, :, 0, :])
            nc.scalar.dma_start(out=whh_sb[:, :, 1, :], in_=whh_r[:, :, 1, :])

            # charge the activation-table load before step 0's tanh
            warm = g_pool.tile([128, 1], f32, name="warm")
            nc.scalar.activation(out=warm, in_=ones_f[:, 0:1], func=AF.Tanh)

            # X: recurrence state + taus, fp16:
            # slots [0:2]=s  [2:4]=tau_g  [4:6]=tau_f  [6:8]=tau_i  [8:10]=tau_o
            X = g_pool.tile([128, 10, BC], bf16)
            nc.vector.memset(X[:, 0:2, :], 0.0)
            W2 = g_pool.tile([128, 4, BC], bf16)
            op1_t = g_pool.tile([128, 2, BC], bf16)
            q_t = g_pool.tile([128, 2, BC], bf16)
            # ping-pong: step t writes slot t%2, the matmuls of step t+1 read it
            t1_t = g_pool.tile([128, 2, 2, BC], bf16)
            nc.vector.memset(t1_t.rearrange("p s k b -> p (s k b)"), 0.0)
            tq_t = g_pool.tile([128, 2, 2, BC], bf16)
            nc.vector.memset(tq_t.rearrange("p s k b -> p (s k b)"), 0.0)

            # lstm_out (h-tilde = 4h) feature-major: [p, k(2 H-chunks), b, t]
            lstm_fm = lstm_pool.tile([128, 2, BC, t_steps], bf16)
            # attention persistents
            attn_t = g_pool.tile([128, 2, BC, t_steps], bf16, tag="attn")
            excl_t = g_pool.tile([128, 2, BC, t_steps], bf16, tag="excl")
            nc.vector.memset(
                excl_t[:, :, :, 0:1].rearrange("p k b o -> p (k b o)"), 0.0)

            # ---- phase-3 weights (off the SP queue: transposes need it) ----
            mlp_sb = p3_pool.tile([128, 2, DA], bf16)
            nc.gpsimd.dma_start(out=mlp_sb,
                                in_=mlpWT.rearrange("(k p) a -> p k a", p=128))
            simrep_sb = p3_pool.tile([128, 2, 128], bf16)
            nc.gpsimd.dma_start(out=simrep_sb,
                                in_=simW.rearrange("(k p) o -> p k o", p=128))
            if has_mlpb:
                mlpb_sb = p3_pool.tile([1, DA], bf16)
                nc.gpsimd.dma_start(out=mlpb_sb, in_=mlpb)
            fc_sb = p3_pool.tile([128, 4, NC], bf16)
            nc.gpsimd.dma_start(out=fc_sb,
                                in_=fcWT.rearrange("(k p) c -> p k c", p=128))
            if has_fcb:
                fcb_sb = p3_pool.tile([1, NC], bf16)
                nc.gpsimd.dma_start(out=fcb_sb, in_=fcb)



            # xg per block: [p, j(8 gate chunks), b, t-in-block]
            xg_tiles = {}

            def xg_tile(blk):
                if blk not in xg_tiles:
                    xg_tiles[blk] = xg_pool.tile(
                        [128, 8, BC, BLK], bf16, tag=f"xg{blk % 2}",
                        name=f"xgb{blk}")
                return xg_tiles[blk]

            xg_loaded = set()

            def emit_xg_dma(blk, parts=2):
                if blk in xg_loaded or blk >= nblk:
                    return
                xg_loaded.add(blk)
                step = BLK // parts
                xt = xg_tile(blk)
                for pi in range(parts):
                    nc.sync.dma_start(
                        out=xt[:, :, :, pi * step:(pi + 1) * step],
                        in_=xg_d[blk, :, :, :, pi * step:(pi + 1) * step])

            emit_xg_dma(0, parts=8)
            emit_xg_dma(1)

            # ============ Phase 3 emission (as thunks) ============
            # Attention pipeline at 64-step granularity; FC per 128-token
            # chunk.  Thunks are dripped a few per step into the engine-idle
            # windows of the recurrence loop.
            att_pool = p3s_pool  # scratch pools rotate via bufs=2
            SUB = 64
            cwr_tiles = {}
            cum_tiles = {}

            def att_thunks(sb):
                """Attention pipeline for t-range [sb*SUB, (sb+1)*SUB)."""
                t0 = sb * SUB
                th = []
                att_n = att_pool.tile([128, 2, BC, SUB], bf16, tag="attn_s",
                                      name=f"attn{sb}")
                wrep = att_pool.tile([128, BC, SUB], bf16, tag="wrep",
                                     name=f"wrep{sb}")
                cwr = att_pool.tile([128, BC, SUB], f32, tag="cwr",
                                    name=f"cwr{sb}")
                rwr = att_pool.tile([128, BC, SUB], f32, tag="rwr",
                                    name=f"rwr{sb}")
                wh = att_pool.tile([128, 2, BC, SUB], bf16, tag="wh",
                                   name=f"wh{sb}")
                cum = att_pool.tile([128, 2, BC, SUB], f32, tag="cum",
                                    name=f"cum{sb}")
                cwr_prev = cwr_tiles.get(sb - 1)
                cum_prev = cum_tiles.get(sb - 1)
                cwr_tiles[sb] = cwr
                cum_tiles[sb] = cum

                # --- att = tanh(mlp_W @ h) ---
                # two shared PSUM tags; mm-group then act-group ordering so
                # dripped acts are ready the moment the Act engine is free.
                def att_mm_group(m, bh, aps):
                    out = []
                    for bi in range(4):
                        bq = bh * 4 + bi
                        for k2 in range(2):
                            def mm(m=m, bi=bi, bq=bq, k2=k2, aps=aps):
                                nc.tensor.matmul(
                                    out=aps[:, bi, :],
                                    lhsT=mlp_sb[:, k2, 128 * m:128 * (m + 1)],
                                    rhs=lstm_fm[:, k2, bq, t0:t0 + SUB],
                                    start=(k2 == 0),
                                    stop=(k2 == 1 and not has_mlpb))
                            out.append(mm)
                        if has_mlpb:
                            def mmb(m=m, bi=bi, aps=aps):
                                nc.tensor.matmul(
                                    out=aps[:, bi, :],
                                    lhsT=mlpb_sb[:, 128 * m:128 * (m + 1)],
                                    rhs=ones_b[0:1, :SUB],
                                    start=False, stop=True)
                            out.append(mmb)
                    return out

                def att_act(m, bh, aps):
                    def act(m=m, bh=bh, aps=aps):
                        nc.scalar.activation(
                            out=att_n[:, m, bh * 4:bh * 4 + 4, :].rearrange(
                                "p b t -> p (b t)"),
                            in_=aps.rearrange("p b t -> p (b t)"),
                            func=AF.Tanh)
                    return act

                # bh0's full chain (att m0, m1 -> score -> exp) first so
                # the bh0 scans can start while bh1's atts still run.
                for bh in range(2):
                    aps0 = ps_att.tile([128, 4, SUB], f32, tag="ab0",
                                       name=f"aps{sb}_0{bh}")
                    th.extend(att_mm_group(0, bh, aps0))
                    aps1 = ps_att.tile([128, 4, SUB], f32, tag="ab1",
                                       name=f"aps{sb}_1{bh}")
                    th.extend(att_mm_group(1, bh, aps1))
                    th.append(att_act(0, bh, aps0))
                    th.append(att_act(1, bh, aps1))
                    bps = ps_att.tile([128, 4, SUB], f32, tag="ab0",
                                      name=f"bps{sb}_{bh}")
                    for bi in range(4):
                        bq = bh * 4 + bi
                        for m in range(2):
                            def mm(bi=bi, bq=bq, m=m, bps=bps):
                                nc.tensor.matmul(
                                    out=bps[:, bi, :],
                                    lhsT=simrep_sb[:, m, :],
                                    rhs=att_n[:, m, bq, :],
                                    start=(m == 0), stop=(m == 1))
                            th.append(mm)
                    def act(bh=bh, bps=bps):
                        nc.scalar.activation(
                            out=wrep[:, bh * 4:bh * 4 + 4, :].rearrange(
                                "p b t -> p (b t)"),
                            in_=bps.rearrange("p b t -> p (b t)"),
                            func=AF.Exp)
                    th.append(act)

                # --- per-sequence scan groups: cwr, recip, wh, cum,
                # attn, excl for one b.  Emitted per-b so the tail can
                # interleave each sequence's FC right after its scans. ---
                n_out = SUB if t0 + SUB < t_steps else SUB - 1
                per_b = []
                for bq in range(BC):
                    g = []
                    def scn_w(bq=bq, sb=sb):
                        init = (0.0 if sb == 0
                                else cwr_prev[:, bq, SUB - 1:SUB])
                        nc.vector.tensor_tensor_scan(
                            out=cwr[:, bq, :], data0=ones_f[:, :SUB],
                            data1=wrep[:, bq, :],
                            initial=init, op0=OP.mult, op1=OP.add)
                    g.append(scn_w)
                    def rcp(bq=bq):
                        nc.vector.reciprocal(
                            out=rwr[:, bq, :], in_=cwr[:, bq, :])
                    g.append(rcp)
                    for k2 in range(2):
                        def mul_wh(k2=k2, bq=bq):
                            nc.gpsimd.tensor_mul(
                                wh[:, k2, bq, :],
                                wrep[:, bq, :],
                                lstm_fm[:, k2, bq, t0:t0 + SUB])
                        g.append(mul_wh)
                    for k2 in range(2):
                        def scn_c(k2=k2, bq=bq, sb=sb):
                            init = (0.0 if sb == 0
                                    else cum_prev[:, k2, bq, SUB - 1:SUB])
                            nc.vector.tensor_tensor_scan(
                                out=cum[:, k2, bq, :],
                                data0=ones_f[:, :SUB],
                                data1=wh[:, k2, bq, :],
                                initial=init, op0=OP.mult, op1=OP.add)
                        g.append(scn_c)
                    for k2 in range(2):
                        def mul_a(k2=k2, bq=bq):
                            nc.gpsimd.tensor_mul(
                                attn_t[:, k2, bq, t0:t0 + SUB],
                                cum[:, k2, bq, :],
                                rwr[:, bq, :])
                        g.append(mul_a)
                    for k2 in range(2):
                        def scn_e(k2=k2, bq=bq, n_out=n_out, t0=t0):
                            nc.vector.tensor_tensor_scan(
                                out=excl_t[:, k2, bq, t0 + 1:t0 + 1 + n_out],
                                data0=ones_f[:, :n_out],
                                data1=attn_t[:, k2, bq, t0:t0 + n_out],
                                initial=excl_t[:, k2, bq, t0:t0 + 1],
                                op0=OP.mult, op1=OP.add)
                        g.append(scn_e)
                    per_b.append(g)
                return th, per_b

            def fc_chunk(blk, bq, t0=None, tlen=BLK, epilogue="act"):
                """FC for one token chunk (bq, [t0, t0+tlen)).  epilogue
                "act" = tanh on Act; "dve" = raw-logit copy on DVE (the host
                applies the tanh for those rows)."""
                if t0 is None:
                    t0 = blk * BLK
                th = []
                ysbs[(bq, t0)] = p3o_pool.tile([tlen, NC], bf16, tag="ysb",
                                               name=f"ysb{t0}_{bq}")
                for half in range(2):
                    ops = ps_out.tile([tlen, 512], f32, tag="ops",
                                      name=f"ops{t0}_{bq}_{half}")
                    srcs = [(excl_t, 0), (excl_t, 1),
                            (lstm_fm, 0), (lstm_fm, 1)]
                    for k4, (src, k2) in enumerate(srcs):
                        def mm(bq=bq, half=half, k4=k4, src=src, k2=k2,
                               ops=ops, t0=t0):
                            nc.tensor.matmul(
                                out=ops,
                                lhsT=src[:, k2, bq, t0:t0 + tlen],
                                rhs=fc_sb[:, k4,
                                          512 * half:512 * (half + 1)],
                                start=(k4 == 0),
                                stop=(k4 == 3 and not has_fcb))
                        th.append(mm)
                    if has_fcb:
                        def mmb(half=half, ops=ops):
                            nc.tensor.matmul(
                                out=ops, lhsT=ones_b[0:1, 0:tlen],
                                rhs=fcb_sb[:, 512 * half:512 * (half + 1)],
                                start=False, stop=True)
                        th.append(mmb)
                    if epilogue == "act":
                        def act(half=half, ops=ops, bq=bq, t0=t0):
                            nc.scalar.activation(
                                out=ysbs[(bq, t0)][:, 512 * half:
                                                   512 * (half + 1)],
                                in_=ops, func=AF.Tanh)
                        th.append(act)
                    else:
                        def cpy(half=half, ops=ops, bq=bq, t0=t0):
                            nc.vector.tensor_copy(
                                out=ysbs[(bq, t0)][:, 512 * half:
                                                   512 * (half + 1)],
                                in_=ops)
                        th.append(cpy)
                def dma(bq=bq, t0=t0):
                    r0 = bq * t_steps + t0
                    nc.sync.dma_start(out=yout[r0:r0 + tlen, :],
                                      in_=ysbs[(bq, t0)])
                th.append(dma)
                return th

            ysbs = {}
            drip = deque()

            # ================= Phase 2: LSTM recurrence ================
            # gate-chunk order in gps: [g, f, i, o] (host perm).  Cell math:
            #   s'  = 0.5*(tau_f+1)*s + (tau_i+1)*tau_g          (s = 2c)
            #   h~  = (tau_o+1) * s' * (1 - s'^2/12)   (~= 4h, cubic tanh)
            for t in range(t_steps):
                blk = t // BLK
                pp, ppn = (t + 1) % 2, t % 2
                gps = ps_g.tile([128, 8, BC], f32, tag="gps")
                # xg preload (off critical path: no h dependency)
                nc.tensor.matmul(
                    out=gps.rearrange("p j b -> p (j b)"),
                    lhsT=ident_b,
                    rhs=xg_tile(blk)[:, :, :, t - blk * BLK].rearrange(
                        "p j b -> p (j b)"),
                    start=True, stop=False)
                # gates += -12*Whh @ t1  (t1 lands ~113ns before tq)
                for j in range(8):
                    for k2 in range(2):
                        nc.tensor.matmul(
                            out=gps[:, j, :],
                            lhsT=whh_sb[:, k2, 0, 128 * j:128 * (j + 1)],
                            rhs=t1_t[:, pp, k2, :],
                            start=False, stop=False)
                # gates += Whh @ (t1*q)   [== Whh @ h-tilde']
                for j in range(8):
                    for k2 in range(2):
                        nc.tensor.matmul(
                            out=gps[:, j, :],
                            lhsT=whh_sb[:, k2, 1, 128 * j:128 * (j + 1)],
                            rhs=tq_t[:, pp, k2, :],
                            start=False, stop=(j == 7 and k2 == 1))
                # one Act: all 8 gate chunks -> taus
                nc.scalar.activation(
                    out=X[:, 2:10, :].rearrange("p j b -> p (j b)"),
                    in_=gps.rearrange("p j b -> p (j b)"),
                    func=AF.Tanh)
                nc.gpsimd.tensor_scalar(
                    out=op1_t.rearrange("p j b -> p (j b)"),
                    in0=X[:, 8:10, :].rearrange("p j b -> p (j b)"),
                    scalar1=1.0, scalar2=1.0, op0=OP.add, op1=OP.mult)
                # STT1: [w1; v] = (X[4:8]+1) * [s; tau_g]
                nc.vector.scalar_tensor_tensor(
                    out=W2.rearrange("p j b -> p (j b)"),
                    in0=X[:, 4:8, :].rearrange("p j b -> p (j b)"),
                    scalar=1.0,
                    in1=X[:, 0:4, :].rearrange("p j b -> p (j b)"),
                    op0=OP.add, op1=OP.mult)
                # STT2: s' = 0.5*w1 + v
                nc.vector.scalar_tensor_tensor(
                    out=X[:, 0:2, :].rearrange("p j b -> p (j b)"),
                    in0=W2[:, 0:2, :].rearrange("p j b -> p (j b)"),
                    scalar=0.5,
                    in1=W2[:, 2:4, :].rearrange("p j b -> p (j b)"),
                    op0=OP.mult, op1=OP.add)
                sfl = X[:, 0:2, :].rearrange("p j b -> p (j b)")
                hout = lstm_fm[:, :, :, t].rearrange("p k b -> p (k b)")
                # tier 5 on Pool: t1 = (tau_o+1)*s' first (tq needs it), then
                # q = s'^2
                qfl = q_t.rearrange("p j b -> p (j b)")
                t1fl = t1_t[:, ppn].rearrange("p k b -> p (k b)")
                tqfl = tq_t[:, ppn].rearrange("p k b -> p (k b)")
                nc.gpsimd.tensor_mul(
                    t1fl, op1_t.rearrange("p j b -> p (j b)"), sfl)
                nc.gpsimd.tensor_mul(qfl, sfl, sfl)
                # tier 6 on Pool (critical): tq = t1*q; the next step's gates
                # use  Whh @ h~' = Whh @ tq - 12*Whh @ t1.
                nc.gpsimd.tensor_mul(tqfl, t1fl, qfl)
                # off the critical cycle: h~' = (q-12)*t1 for attention/FC
                nc.vector.scalar_tensor_tensor(
                    out=hout, in0=qfl, scalar=-12.0, in1=t1fl,
                    op0=OP.add, op1=OP.mult)

                # ---- prefetch next xg block (resource-gated by xg slot) ----
                if t % BLK == 0 and t > 0:
                    emit_xg_dma(t // BLK + 1)
                # ---- drip: phase-3 thunks ----
                if t % 64 == 63 and t < t_steps - 1:
                    sb = t // 64
                    pre, per_b = att_thunks(sb)
                    drip.extend(pre)
                    for bq in range(BC):
                        drip.extend(per_b[bq])
                if t % BLK == BLK - 1 and t < t_steps - 1 and blk < nblk - 1:
                    for bq in range(BC):
                        drip.extend(fc_chunk(blk, bq))
                if drip:
                    quota = max(3, (len(drip) + 55) // 56)
                    for _ in range(min(quota, len(drip))):
                        drip.popleft()()

            # tail: last sub-block's attention per-b, each followed by the
            # sequence's FC chunk; tiny warmup matmuls keep the PE p-state hot.
            while drip:
                drip.popleft()()
            warm_ps = ps_att.tile([128, 4, SUB], f32, tag="ab0", name="warmps")

            def pe_warm():
                nc.tensor.matmul(out=warm_ps[:, 0, 0:1], lhsT=ident_b,
                                 rhs=ones_b[:, 0:1], start=True, stop=True)

            last_sb = t_steps // 64 - 1
            pre, per_b = att_thunks(last_sb)
            for f in pre:
                f()
            pe_warm()
            for bq in range(BC):
                for f in per_b[bq]:
                    f()
                pe_warm()
                for f in fc_chunk(nblk - 1, bq,
                                  epilogue=("act" if bq % 2 == 0 else "dve")):
                    f()

    return nc


# ----------------------------------------------------------------------------
# Host-side weight preparation
# ----------------------------------------------------------------------------

def _prepare(inputs):
    W_ih = inputs["W_ih"].astype(np.float64)
    W_hh = inputs["W_hh"].astype(np.float64)
    b_ih = inputs["b_ih"].astype(np.float64)
    b_hh = inputs["b_hh"].astype(np.float64)
    ec = inputs["embed_concept"].astype(np.float64)
    er = inputs["embed_correct"].astype(np.float64)

    W_A = W_ih[:, :DC]
    W_B = W_ih[:, DC:]
    bias = b_ih + b_hh
    # T[0*NC + cid] : corr=0 -> inter=[v0; u]  => W_A v0 + W_B u + bias
    # T[1*NC + cid] : corr=1 -> inter=[u; v1]  => W_A u + W_B v1 + bias
    T0 = ec @ W_B.T + (W_A @ er[0] + bias)[None, :]
    T1 = ec @ W_A.T + (W_B @ er[1] + bias)[None, :]
    Tbl = np.concatenate([T0, T1], axis=0)

    # device gate order [g, f, i, o]; i,f,o preacts halved so that
    # sigma(a) = 0.5*tanh(a/2)+0.5 becomes 0.5*(tau+1)
    perm = np.concatenate([np.arange(2 * DL, 3 * DL),   # g
                           np.arange(DL, 2 * DL),       # f
                           np.arange(0, DL),            # i
                           np.arange(3 * DL, 4 * DL)])  # o
    beta = np.concatenate([np.full(DL, 1.0),            # g
                           np.full(3 * DL, 0.5)])       # f, i, o
    Tbl = (Tbl[:, perm] * beta[None, :])
    # lstm_fm holds h-tilde' = -12 * 4h = -48h  (the device computes
    # (q-12)*t1 = -12*h~) -> all consumers of lstm_fm divide by -48.
    HS = -48.0
    Whh_eff = (W_hh[perm] * beta[:, None]) / HS
    # gates use Whh @ h~' = Whh @ tq - 12 * Whh @ t1 (tq = t1*q on device)
    WhhT2 = np.stack([-12.0 * Whh_eff.T, Whh_eff.T], axis=1)  # [DL, 2, 4DL]

    bf = np.float16
    return {
        "tbl": np.ascontiguousarray(Tbl).astype(bf),
        "whhT": np.ascontiguousarray(WhhT2).astype(bf),
        "mlpWT": np.ascontiguousarray(inputs["mlp_W"].astype(np.float64).T / HS).astype(bf),
        "mlpb": np.ascontiguousarray(inputs["mlp_b"][None, :]).astype(bf),
        "simW": np.ascontiguousarray(
            np.tile(inputs["sim_W"].reshape(DA, 1), (1, 128))).astype(bf),
        "fcWT": np.ascontiguousarray(
            inputs["fc_W"].astype(np.float64).T / (2.0 * HS)).astype(bf),
        "fcb": np.ascontiguousarray(inputs["fc_b"][None, :] / 2.0).astype(bf),
    }


_CACHE = {}


def kernel(**inputs):
    from concourse.bass_utils import run_bass_kernel_spmd

    has_mlpb = bool(np.any(inputs["mlp_b"] != 0))
    has_fcb = bool(np.any(inputs["fc_b"] != 0))
    key = ("nc", has_mlpb, has_fcb)
    if key not in _CACHE:
        _CACHE[key] = build_kernel(has_mlpb=has_mlpb, has_fcb=has_fcb)
    nc = _CACHE[key]

    shared = _prepare(inputs)
    tbl = shared.pop("tbl")
    cseq = inputs["concept_seq"].astype(np.int64)
    rseq = inputs["correct_seq"].astype(np.int64)
    idx = rseq * NC + cseq                                  # [B, T]
    xg_all = tbl[idx]                                       # [B, T, 8*128]
    # -> [B, nblk, tb, j, p] -> per core [nblk, p, j, b, tb]
    xg_all = xg_all.reshape(B, T // 128, 128, 8, 128)

    in_maps = []
    for i in range(N_CORES):
        m = dict(shared)
        xgc = xg_all[i * BC:(i + 1) * BC]                   # [BC, blk, tb, j, p]
        m["xg"] = np.ascontiguousarray(xgc.transpose(1, 4, 3, 0, 2))
        in_maps.append(m)

    res = run_bass_kernel_spmd(nc, in_maps, list(range(N_CORES)))
    out = np.concatenate(
        [np.asarray(res.results[i]["y"]).astype(np.float32).reshape(BC, T, NC)
         for i in range(N_CORES)], axis=0)
    # odd sequences' last-block rows were emitted as raw logits (their
    # device epilogue was a DVE copy, not a tanh) -- finish them here
    raw = out[:, T - BLK:, :]
    raw[1::2] = np.tanh(raw[1::2])
    return out * 0.5 + 0.5


# revision 54
# speedup vs baseline: 1.0003x; 1.0003x over previous
"""ATKT (LSTM + degenerate causal attention + FC) Trainium2 kernel, v3.

Full inputs in, full outputs out. Shards batch (64) across 8 NeuronCores
(8 sequences per core), runs a Bass/Tile kernel per core, reassembles.

The Tile scheduler's cost model charges ~100ns (SEM_DELAY) per data-
dependency edge, so the LSTM recurrence (512 serial steps) is bound by
the number of serial edges per step, not engine throughput. v3 cuts the
baseline's 8-edge step to 6 edges:

  Whh-matmuls -> tanh(all 8 gate chunks, one Act) -> STT1 -> STT2
      -> {t1 = (tau_o+1)*s', q = s'^2 on Pool} -> tq = t1*q (Pool)

and feeds the PE *two* rhs streams instead of h:
  gates = xg + Whh @ h~' = xg - 12*Whh @ t1 + Whh @ (t1*q)
(the cubic-tanh h~' = (q-12)*t1 = -12*h~ is materialized off the
critical cycle for attention/FC only; the -1/12 scale is folded into
all weight consumers on the host).

Other restructurings:
 - xg (the interaction-embedding gate table) is gathered AND transposed
   on the host, so phase 1 is four plain DMAs instead of 32 indirect
   gathers + 32 DMA transposes on engine queues.
 - Attention (64-step granularity) and FC (128-token chunks) are
   emitted as thunks and dripped a few per step into the engine-idle
   windows of the recurrence loop; the last sub-block's pipeline is
   ordered per-sequence (scans then that sequence's FC) to minimize the
   tail.
 - The final sigmoid is computed as tanh on-device (table shared with
   the recurrence) and 0.5*y+0.5 on the host; y is fp16.
"""
import os
import sys
from collections import deque

sys.path.insert(0, "/opt/trn_rl_repo")

import numpy as np
import ml_dtypes

B, T = 64, 512
DC = DR = DL = DA = 256
NC = 1024
N_CORES = 8
BC = B // N_CORES          # sequences per core
TOK = BC * T               # tokens per core (4096)
BLK = 128                  # recurrence block size for phase-3 streaming
NBLK = T // BLK

# Custom DVE ops and gpsimd-STT both fail this container's walrus codegen
# ("ISA wrong length" / "engine check failed") -- stock ops only.
USE_CUSTOM_DVE = False
STT_ON_POOL = False

# ----------------------------------------------------------------------------
# Walrus workaround: this container's neuronxcc rejects >1 sync wait per
# instruction ("Too many sync wait commands"). Split multi-wait instructions
# into single-wait NoOps on the same engine.
# ----------------------------------------------------------------------------


def _apply_tile_patches():
    import bass_rust
    import concourse.tile as tile
    from concourse import mybir

    if getattr(tile.TileContext, "_waitsplit_patched", False):
        return

    _orig_lower = tile.TileContext._lower_ordered_insts

    def _split_waits_in_list(uid, insts, counter):
        new_list = []
        for inst in insts:
            si = inst.sync_info
            if si is not None and len(si.on_wait) > 1:
                waits = list(si.on_wait)
                for w in waits[:-1]:
                    counter[0] += 1
                    nop = mybir.InstNoOp(
                        name=f"waitsplit_{uid}_{counter[0]}",
                        engine=inst.engine,
                        sync_info=bass_rust.SyncInfo(on_wait=[w], on_update=[]),
                        bass_nofuse=True,
                    )
                    new_list.append(nop)
                inst.sync_info = bass_rust.SyncInfo(
                    on_wait=[waits[-1]], on_update=list(si.on_update))
            new_list.append(inst)
        return new_list

    def _patched_lower(self, ordered):
        counter = [0]
        for bb_name in list(ordered.keys()):
            ordered[bb_name] = _split_waits_in_list(self.uid, ordered[bb_name], counter)
        return _orig_lower(self, ordered)

    def _patched_drain_and_barrier(self, tick_clock, wait_clock):
        nc = self.nc
        drain_inst = nc.sync.drain()
        wait_clock.add_sem_waits(
            drain_inst.ins, tile.ScopedClock({None: tick_clock.global_clock}))
        si = drain_inst.ins.sync_info
        if si is not None and len(si.on_wait) > 1:
            waits = list(si.on_wait)
            drain_inst.ins.sync_info = bass_rust.SyncInfo(
                on_wait=waits[:1], on_update=list(si.on_update))
            for w in waits[1:]:
                nop = nc.sync.nop(nofuse=True)
                nop.ins.sync_info = bass_rust.SyncInfo(on_wait=[w], on_update=[])
        nc.all_engine_barrier()
        assert self.sems is not None
        popped = nc._tile_sem_poison_stack.pop()
        assert popped is self._sem_poison
        nc.clear_and_free_semaphores(list(self.sems.allocated().values()))
        nc.all_engine_barrier()

    tile.TileContext._lower_ordered_insts = _patched_lower
    tile.TileContext._drain_and_barrier = _patched_drain_and_barrier
    tile.TileContext._waitsplit_patched = True


# ----------------------------------------------------------------------------
# Custom DVE op: h~ = (tau_o + 1) * s * (1 + C2 * s^2), C2 = -1/12.
# Registered through the documented dve_ops extension point (OPS list);
# the uop program ships in the per-NEFF DVE table, no firmware change.
# ----------------------------------------------------------------------------

_HTILDE = None


def _register_htilde():
    global _HTILDE
    if _HTILDE is not None:
        return _HTILDE
    import concourse.dve_ops as dve_ops
    from concourse.dve_ops import DveOp, OPS, CUSTOM_DVE_SPECS, _SUB_OPCODE_FOR_NAME
    from concourse.dve_spec import Spec, Src0, Src1, C2, One, sq, lower
    from concourse.dve_uop import DveOpSpec, DveVer

    name = "ATKT_HTILDE"
    if name in _SUB_OPCODE_FOR_NAME:
        _HTILDE = next(op for op in OPS if op.name == name)
        return _HTILDE

    body = (Src0 + One) * Src1 * (sq(Src1) * C2 + One)
    spec = Spec(
        body=body,
        reference=lambda in0, in1, s0, s1, imm2: (
            (in0.astype(np.float32) + 1.0)
            * in1.astype(np.float32)
            * (np.square(in1.astype(np.float32)) * imm2 + 1.0)
        ),
    )
    row = max(_SUB_OPCODE_FOR_NAME.values()) + 1
    assert row < 0x20
    _SUB_OPCODE_FOR_NAME[name] = row

    shas = {}
    for ver in ("v3", "v4"):
        uops = lower(spec, ver=ver)
        tmp = DveOpSpec(name=name, opcode=row, uops=uops, rd1_en=True)
        shas[ver] = tmp.sha(ver)
    op = DveOp(name, spec, subdim=False, uops_sha=shas)
    OPS.append(op)
    CUSTOM_DVE_SPECS[name] = spec
    _HTILDE = op
    return op


# ----------------------------------------------------------------------------
# Kernel build
# ----------------------------------------------------------------------------

def build_kernel(t_steps=T, has_mlpb=False, has_fcb=False):
    import concourse.bass as bass
    import concourse.tile as tile
    from concourse import mybir

    _apply_tile_patches()
    htilde_op = _register_htilde() if USE_CUSTOM_DVE else None

    f32 = mybir.dt.float32
    # fp16: |attn_excl| reaches ~320 and fc logits ~43; bf16's 8-bit mantissa
    # costs 4e-2 output error, fp16's 10 bits is fine.
    bf16 = mybir.dt.float16
    i32 = mybir.dt.int32
    AF = mybir.ActivationFunctionType
    OP = mybir.AluOpType

    nc = bass.Bass("TRN2", target_bir_lowering=False, debug=False,
                   num_devices=N_CORES)

    n_tok = BC * t_steps
    n_tc = n_tok // 128            # 128-token chunks
    tc_per_seq = t_steps // 128
    nblk = t_steps // BLK

    # ---- DRAM parameters (per core) ----
    # xg is gathered AND transposed on the host: [blk, p, j, b, t-in-block]
    xg_d = nc.dram_tensor(
        "xg", [t_steps // BLK, 128, 8, BC, BLK], bf16,
        kind="ExternalInput").ap()
    whhT = nc.dram_tensor("whhT", [DL, 2, 4 * DL], bf16, kind="ExternalInput").ap()
    mlpWT = nc.dram_tensor("mlpWT", [DL, DA], bf16, kind="ExternalInput").ap()
    mlpb = nc.dram_tensor("mlpb", [1, DA], bf16, kind="ExternalInput").ap()
    simW = nc.dram_tensor("simW", [DA, 128], bf16, kind="ExternalInput").ap()
    fcWT = nc.dram_tensor("fcWT", [2 * DL, NC], bf16, kind="ExternalInput").ap()
    fcb = nc.dram_tensor("fcb", [1, NC], bf16, kind="ExternalInput").ap()
    yout = nc.dram_tensor("y", [n_tok, NC], bf16, kind="ExternalOutput").ap()

    with tile.TileContext(nc) as tc:
        import contextlib
        with contextlib.ExitStack() as ctx:
            g_pool = ctx.enter_context(tc.tile_pool(name="globals", bufs=1))
            lstm_pool = ctx.enter_context(tc.tile_pool(name="lstm", bufs=1))
            xg_pool = ctx.enter_context(tc.tile_pool(name="xg", bufs=1))
            row_pool = ctx.enter_context(tc.tile_pool(name="rows", bufs=6))
            p3_pool = ctx.enter_context(tc.tile_pool(name="p3", bufs=1))
            p3s_pool = ctx.enter_context(tc.tile_pool(name="p3scratch", bufs=2))
            p3o_pool = ctx.enter_context(tc.tile_pool(name="p3out", bufs=2))
            ps_g = ctx.enter_context(
                tc.tile_pool(name="ps_gates", bufs=2, space="PSUM"))
            ps_att = ctx.enter_context(
                tc.tile_pool(name="ps_att", bufs=1, space="PSUM"))
            ps_out = ctx.enter_context(
                tc.tile_pool(name="ps_out", bufs=4, space="PSUM"))

            # ---- persistent small tiles ----
            ones_f = g_pool.tile([128, 512], f32)
            nc.vector.memset(ones_f, 1.0)
            ones_b = g_pool.tile([128, 512], bf16)
            nc.vector.memset(ones_b, 1.0)
            ident_b = g_pool.tile([128, 128], bf16)
            nc.vector.memset(ident_b, 1.0)
            nc.gpsimd.affine_select(
                out=ident_b, in_=ident_b, pattern=[[-1, 128]],
                compare_op=OP.is_equal, fill=0.0, base=0, channel_multiplier=1)

            whh_sb = g_pool.tile([128, 2, 2, 4 * DL], bf16)
            whh_r = whhT.rearrange("(k p) s g -> p k s g", p=128)
            # split across two queues so neither DMA alone gates step 0
            nc.gpsimd.dma_start(out=whh_sb[:, :, 0, :], in_=whh_r[:, :, 0, :])
            nc.scalar.dma_start(out=whh_sb[:, :, 1, :], in_=whh_r[:, :, 1, :])

            # charge the activation-table load before step 0's tanh
            warm = g_pool.tile([128, 1], f32, name="warm")
            nc.scalar.activation(out=warm, in_=ones_f[:, 0:1], func=AF.Tanh)

            # X: recurrence state + taus, fp16:
            # slots [0:2]=s  [2:4]=tau_g  [4:6]=tau_f  [6:8]=tau_i  [8:10]=tau_o
            X = g_pool.tile([128, 10, BC], bf16)
            nc.vector.memset(X[:, 0:2, :], 0.0)
            W2 = g_pool.tile([128, 4, BC], bf16)
            op1_t = g_pool.tile([128, 2, BC], bf16)
            q_t = g_pool.tile([128, 2, BC], bf16)
            # ping-pong: step t writes slot t%2, the matmuls of step t+1 read it
            t1_t = g_pool.tile([128, 2, 2, BC], bf16)
            nc.vector.memset(t1_t.rearrange("p s k b -> p (s k b)"), 0.0)
            tq_t = g_pool.tile([128, 2, 2, BC], bf16)
            nc.vector.memset(tq_t.rearrange("p s k b -> p (s k b)"), 0.0)

            # lstm_out (h-tilde = 4h) feature-major: [p, k(2 H-chunks), b, t]
            lstm_fm = lstm_pool.tile([128, 2, BC, t_steps], bf16)
            # attention persistents
            attn_t = g_pool.tile([128, 2, BC, t_steps], bf16, tag="attn")
            excl_t = g_pool.tile([128, 2, BC, t_steps], bf16, tag="excl")
            nc.vector.memset(
                excl_t[:, :, :, 0:1].rearrange("p k b o -> p (k b o)"), 0.0)

            # ---- phase-3 weights (off the SP queue: transposes need it) ----
            mlp_sb = p3_pool.tile([128, 2, DA], bf16)
            nc.gpsimd.dma_start(out=mlp_sb,
                                in_=mlpWT.rearrange("(k p) a -> p k a", p=128))
            simrep_sb = p3_pool.tile([128, 2, 128], bf16)
            nc.gpsimd.dma_start(out=simrep_sb,
                                in_=simW.rearrange("(k p) o -> p k o", p=128))
            if has_mlpb:
                mlpb_sb = p3_pool.tile([1, DA], bf16)
                nc.gpsimd.dma_start(out=mlpb_sb, in_=mlpb)
            fc_sb = p3_pool.tile([128, 4, NC], bf16)
            nc.gpsimd.dma_start(out=fc_sb,
                                in_=fcWT.rearrange("(k p) c -> p k c", p=128))
            if has_fcb:
                fcb_sb = p3_pool.tile([1, NC], bf16)
                nc.gpsimd.dma_start(out=fcb_sb, in_=fcb)



            # xg per block: [p, j(8 gate chunks), b, t-in-block]
            xg_tiles = {}

            def xg_tile(blk):
                if blk not in xg_tiles:
                    xg_tiles[blk] = xg_pool.tile(
                        [128, 8, BC, BLK], bf16, tag=f"xg{blk % 2}",
                        name=f"xgb{blk}")
                return xg_tiles[blk]

            xg_loaded = set()

            def emit_xg_dma(blk, parts=2):
                if blk in xg_loaded or blk >= nblk:
                    return
                xg_loaded.add(blk)
                step = BLK // parts
                xt = xg_tile(blk)
                for pi in range(parts):
                    nc.sync.dma_start(
                        out=xt[:, :, :, pi * step:(pi + 1) * step],
                        in_=xg_d[blk, :, :, :, pi * step:(pi + 1) * step])

            emit_xg_dma(0, parts=8)
            emit_xg_dma(1)

            # ============ Phase 3 emission (as thunks) ============
            # Attention pipeline at 64-step granularity; FC per 128-token
            # chunk.  Thunks are dripped a few per step into the engine-idle
            # windows of the recurrence loop.
            att_pool = p3s_pool  # scratch pools rotate via bufs=2
            SUB = 64
            cwr_tiles = {}
            cum_tiles = {}

            def att_thunks(sb):
                """Attention pipeline for t-range [sb*SUB, (sb+1)*SUB)."""
                t0 = sb * SUB
                th = []
                att_n = att_pool.tile([128, 2, BC, SUB], bf16, tag="attn_s",
                                      name=f"attn{sb}")
                wrep = att_pool.tile([128, BC, SUB], bf16, tag="wrep",
                                     name=f"wrep{sb}")
                cwr = att_pool.tile([128, BC, SUB], f32, tag="cwr",
                                    name=f"cwr{sb}")
                rwr = att_pool.tile([128, BC, SUB], f32, tag="rwr",
                                    name=f"rwr{sb}")
                wh = att_pool.tile([128, 2, BC, SUB], bf16, tag="wh",
                                   name=f"wh{sb}")
                cum = att_pool.tile([128, 2, BC, SUB], f32, tag="cum",
                                    name=f"cum{sb}")
                cwr_prev = cwr_tiles.get(sb - 1)
                cum_prev = cum_tiles.get(sb - 1)
                cwr_tiles[sb] = cwr
                cum_tiles[sb] = cum

                # --- att = tanh(mlp_W @ h) ---
                # two shared PSUM tags; mm-group then act-group ordering so
                # dripped acts are ready the moment the Act engine is free.
                def att_mm_group(m, bh, aps):
                    out = []
                    for bi in range(4):
                        bq = bh * 4 + bi
                        for k2 in range(2):
                            def mm(m=m, bi=bi, bq=bq, k2=k2, aps=aps):
                                nc.tensor.matmul(
                                    out=aps[:, bi, :],
                                    lhsT=mlp_sb[:, k2, 128 * m:128 * (m + 1)],
                                    rhs=lstm_fm[:, k2, bq, t0:t0 + SUB],
                                    start=(k2 == 0),
                                    stop=(k2 == 1 and not has_mlpb))
                            out.append(mm)
                        if has_mlpb:
                            def mmb(m=m, bi=bi, aps=aps):
                                nc.tensor.matmul(
                                    out=aps[:, bi, :],
                                    lhsT=mlpb_sb[:, 128 * m:128 * (m + 1)],
                                    rhs=ones_b[0:1, :SUB],
                                    start=False, stop=True)
                            out.append(mmb)
                    return out

                def att_act(m, bh, aps):
                    def act(m=m, bh=bh, aps=aps):
                        nc.scalar.activation(
                            out=att_n[:, m, bh * 4:bh * 4 + 4, :].rearrange(
                                "p b t -> p (b t)"),
                            in_=aps.rearrange("p b t -> p (b t)"),
                            func=AF.Tanh)
                    return act

                # bh0's full chain (att m0, m1 -> score -> exp) first so
                # the bh0 scans can start while bh1's atts still run.
                for bh in range(2):
                    aps0 = ps_att.tile([128, 4, SUB], f32, tag="ab0",
                                       name=f"aps{sb}_0{bh}")
                    th.extend(att_mm_group(0, bh, aps0))
                    aps1 = ps_att.tile([128, 4, SUB], f32, tag="ab1",
                                       name=f"aps{sb}_1{bh}")
                    th.extend(att_mm_group(1, bh, aps1))
                    th.append(att_act(0, bh, aps0))
                    th.append(att_act(1, bh, aps1))
                    bps = ps_att.tile([128, 4, SUB], f32, tag="ab0",
                                      name=f"bps{sb}_{bh}")
                    for bi in range(4):
                        bq = bh * 4 + bi
                        for m in range(2):
                            def mm(bi=bi, bq=bq, m=m, bps=bps):
                                nc.tensor.matmul(
                                    out=bps[:, bi, :],
                                    lhsT=simrep_sb[:, m, :],
                                    rhs=att_n[:, m, bq, :],
                                    start=(m == 0), stop=(m == 1))
                            th.append(mm)
                    def act(bh=bh, bps=bps):
                        nc.scalar.activation(
                            out=wrep[:, bh * 4:bh * 4 + 4, :].rearrange(
                                "p b t -> p (b t)"),
                            in_=bps.rearrange("p b t -> p (b t)"),
                            func=AF.Exp)
                    th.append(act)

                # --- per-sequence scan groups: cwr, recip, wh, cum,
                # attn, excl for one b.  Emitted per-b so the tail can
                # interleave each sequence's FC right after its scans. ---
                n_out = SUB if t0 + SUB < t_steps else SUB - 1
                per_b = []
                for bq in range(BC):
                    g = []
                    def scn_w(bq=bq, sb=sb):
                        init = (0.0 if sb == 0
                                else cwr_prev[:, bq, SUB - 1:SUB])
                        nc.vector.tensor_tensor_scan(
                            out=cwr[:, bq, :], data0=ones_f[:, :SUB],
                            data1=wrep[:, bq, :],
                            initial=init, op0=OP.mult, op1=OP.add)
                    g.append(scn_w)
                    def rcp(bq=bq):
                        nc.vector.reciprocal(
                            out=rwr[:, bq, :], in_=cwr[:, bq, :])
                    g.append(rcp)
                    for k2 in range(2):
                        def mul_wh(k2=k2, bq=bq):
                            nc.gpsimd.tensor_mul(
                                wh[:, k2, bq, :],
                                wrep[:, bq, :],
                                lstm_fm[:, k2, bq, t0:t0 + SUB])
                        g.append(mul_wh)
                    for k2 in range(2):
                        def scn_c(k2=k2, bq=bq, sb=sb):
                            init = (0.0 if sb == 0
                                    else cum_prev[:, k2, bq, SUB - 1:SUB])
                            nc.vector.tensor_tensor_scan(
                                out=cum[:, k2, bq, :],
                                data0=ones_f[:, :SUB],
                                data1=wh[:, k2, bq, :],
                                initial=init, op0=OP.mult, op1=OP.add)
                        g.append(scn_c)
                    for k2 in range(2):
                        def mul_a(k2=k2, bq=bq):
                            nc.gpsimd.tensor_mul(
                                attn_t[:, k2, bq, t0:t0 + SUB],
                                cum[:, k2, bq, :],
                                rwr[:, bq, :])
                        g.append(mul_a)
                    for k2 in range(2):
                        def scn_e(k2=k2, bq=bq, n_out=n_out, t0=t0):
                            nc.vector.tensor_tensor_scan(
                                out=excl_t[:, k2, bq, t0 + 1:t0 + 1 + n_out],
                                data0=ones_f[:, :n_out],
                                data1=attn_t[:, k2, bq, t0:t0 + n_out],
                                initial=excl_t[:, k2, bq, t0:t0 + 1],
                                op0=OP.mult, op1=OP.add)
                        g.append(scn_e)
                    per_b.append(g)
                return th, per_b

            def fc_chunk(blk, bq, t0=None, tlen=BLK, epilogue="act"):
                """FC for one token chunk (bq, [t0, t0+tlen)).  epilogue
                "act" = tanh on Act; "dve" = raw-logit copy on DVE (the host
                applies the tanh for those rows)."""
                if t0 is None:
                    t0 = blk * BLK
                th = []
                ysbs[(bq, t0)] = p3o_pool.tile([tlen, NC], bf16, tag="ysb",
                                               name=f"ysb{t0}_{bq}")
                for half in range(2):
                    ops = ps_out.tile([tlen, 512], f32, tag="ops",
                                      name=f"ops{t0}_{bq}_{half}")
                    srcs = [(excl_t, 0), (excl_t, 1),
                            (lstm_fm, 0), (lstm_fm, 1)]
                    for k4, (src, k2) in enumerate(srcs):
                        def mm(bq=bq, half=half, k4=k4, src=src, k2=k2,
                               ops=ops, t0=t0):
                            nc.tensor.matmul(
                                out=ops,
                                lhsT=src[:, k2, bq, t0:t0 + tlen],
                                rhs=fc_sb[:, k4,
                                          512 * half:512 * (half + 1)],
                                start=(k4 == 0),
                                stop=(k4 == 3 and not has_fcb))
                        th.append(mm)
                    if has_fcb:
                        def mmb(half=half, ops=ops):
                            nc.tensor.matmul(
                                out=ops, lhsT=ones_b[0:1, 0:tlen],
                                rhs=fcb_sb[:, 512 * half:512 * (half + 1)],
                                start=False, stop=True)
                        th.append(mmb)
                    if epilogue == "act":
                        def act(half=half, ops=ops, bq=bq, t0=t0):
                            nc.scalar.activation(
                                out=ysbs[(bq, t0)][:, 512 * half:
                                                   512 * (half + 1)],
                                in_=ops, func=AF.Tanh)
                        th.append(act)
                    else:
                        def cpy(half=half, ops=ops, bq=bq, t0=t0):
                            nc.vector.tensor_copy(
                                out=ysbs[(bq, t0)][:, 512 * half:
                                                   512 * (half + 1)],
                                in_=ops)
                        th.append(cpy)
                def dma(bq=bq, t0=t0):
                    r0 = bq * t_steps + t0
                    nc.sync.dma_start(out=yout[r0:r0 + tlen, :],
                                      in_=ysbs[(bq, t0)])
                th.append(dma)
                return th

            ysbs = {}
            drip = deque()

            # ================= Phase 2: LSTM recurrence ================
            # gate-chunk order in gps: [g, f, i, o] (host perm).  Cell math:
            #   s'  = 0.5*(tau_f+1)*s + (tau_i+1)*tau_g          (s = 2c)
            #   h~  = (tau_o+1) * s' * (1 - s'^2/12)   (~= 4h, cubic tanh)
            for t in range(t_steps):
                blk = t // BLK
                pp, ppn = (t + 1) % 2, t % 2
                gps = ps_g.tile([128, 8, BC], f32, tag="gps")
                # xg preload (off critical path: no h dependency)
                nc.tensor.matmul(
                    out=gps.rearrange("p j b -> p (j b)"),
                    lhsT=ident_b,
                    rhs=xg_tile(blk)[:, :, :, t - blk * BLK].rearrange(
                        "p j b -> p (j b)"),
                    start=True, stop=False)
                # gates += -12*Whh @ t1  (t1 lands ~113ns before tq)
                for j in range(8):
                    for k2 in range(2):
                        nc.tensor.matmul(
                            out=gps[:, j, :],
                            lhsT=whh_sb[:, k2, 0, 128 * j:128 * (j + 1)],
                            rhs=t1_t[:, pp, k2, :],
                            start=False, stop=False)
                # gates += Whh @ (t1*q)   [== Whh @ h-tilde']
                for j in range(8):
                    for k2 in range(2):
                        nc.tensor.matmul(
                            out=gps[:, j, :],
                            lhsT=whh_sb[:, k2, 1, 128 * j:128 * (j + 1)],
                            rhs=tq_t[:, pp, k2, :],
                            start=False, stop=(j == 7 and k2 == 1))
                # one Act: all 8 gate chunks -> taus
                nc.scalar.activation(
                    out=X[:, 2:10, :].rearrange("p j b -> p (j b)"),
                    in_=gps.rearrange("p j b -> p (j b)"),
                    func=AF.Tanh)
                nc.gpsimd.tensor_scalar(
                    out=op1_t.rearrange("p j b -> p (j b)"),
                    in0=X[:, 8:10, :].rearrange("p j b -> p (j b)"),
                    scalar1=1.0, scalar2=1.0, op0=OP.add, op1=OP.mult)
                # STT1: [w1; v] = (X[4:8]+1) * [s; tau_g]
                nc.vector.scalar_tensor_tensor(
                    out=W2.rearrange("p j b -> p (j b)"),
                    in0=X[:, 4:8, :].rearrange("p j b -> p (j b)"),
                    scalar=1.0,
                    in1=X[:, 0:4, :].rearrange("p j b -> p (j b)"),
                    op0=OP.add, op1=OP.mult)
                # STT2: s' = 0.5*w1 + v
                nc.vector.scalar_tensor_tensor(
                    out=X[:, 0:2, :].rearrange("p j b -> p (j b)"),
                    in0=W2[:, 0:2, :].rearrange("p j b -> p (j b)"),
                    scalar=0.5,
                    in1=W2[:, 2:4, :].rearrange("p j b -> p (j b)"),
                    op0=OP.mult, op1=OP.add)
                sfl = X[:, 0:2, :].rearrange("p j b -> p (j b)")
                hout = lstm_fm[:, :, :, t].rearrange("p k b -> p (k b)")
                # tier 5 on Pool: t1 = (tau_o+1)*s' first (tq needs it), then
                # q = s'^2
                qfl = q_t.rearrange("p j b -> p (j b)")
                t1fl = t1_t[:, ppn].rearrange("p k b -> p (k b)")
                tqfl = tq_t[:, ppn].rearrange("p k b -> p (k b)")
                nc.gpsimd.tensor_mul(
                    t1fl, op1_t.rearrange("p j b -> p (j b)"), sfl)
                nc.gpsimd.tensor_mul(qfl, sfl, sfl)
                # tier 6 on Pool (critical): tq = t1*q; the next step's gates
                # use  Whh @ h~' = Whh @ tq - 12*Whh @ t1.
                nc.gpsimd.tensor_mul(tqfl, t1fl, qfl)
                # off the critical cycle: h~' = (q-12)*t1 for attention/FC
                nc.vector.scalar_tensor_tensor(
                    out=hout, in0=qfl, scalar=-12.0, in1=t1fl,
                    op0=OP.add, op1=OP.mult)

                # ---- prefetch next xg block (resource-gated by xg slot) ----
                if t % BLK == 0 and t > 0:
                    emit_xg_dma(t // BLK + 1)
                # ---- drip: phase-3 thunks ----
                if t % 64 == 63 and t < t_steps - 1:
                    sb = t // 64
                    pre, per_b = att_thunks(sb)
                    drip.extend(pre)
                    for bq in range(BC):
                        drip.extend(per_b[bq])
                if t % BLK == BLK - 1 and t < t_steps - 1 and blk < nblk - 1:
                    for bq in range(BC):
                        drip.extend(fc_chunk(blk, bq))
                if drip:
                    quota = max(3, (len(drip) + 55) // 56)
                    for _ in range(min(quota, len(drip))):
                        drip.popleft()()

            # tail: last sub-block's attention per-b, each followed by the
            # sequence's FC chunk; tiny warmup matmuls keep the PE p-state hot.
            while drip:
                drip.popleft()()
            warm_ps = ps_att.tile([128, 4, SUB], f32, tag="ab0", name="warmps")

            def pe_warm():
                nc.tensor.matmul(out=warm_ps[:, 0, 0:1], lhsT=ident_b,
                                 rhs=ones_b[:, 0:1], start=True, stop=True)

            last_sb = t_steps // 64 - 1
            pre, per_b = att_thunks(last_sb)
            for f in pre:
                f()
            pe_warm()
            for bq in range(BC):
                for f in per_b[bq]:
                    f()
                pe_warm()
                for f in fc_chunk(nblk - 1, bq,
                                  epilogue=("act" if bq < 5 else "dve")):
                    f()

    return nc


# ----------------------------------------------------------------------------
# Host-side weight preparation
# ----------------------------------------------------------------------------

def _prepare(inputs):
    W_ih = inputs["W_ih"].astype(np.float64)
    W_hh = inputs["W_hh"].astype(np.float64)
    b_ih = inputs["b_ih"].astype(np.float64)
    b_hh = inputs["b_hh"].astype(np.float64)
    ec = inputs["embed_concept"].astype(np.float64)
    er = inputs["embed_correct"].astype(np.float64)

    W_A = W_ih[:, :DC]
    W_B = W_ih[:, DC:]
    bias = b_ih + b_hh
    # T[0*NC + cid] : corr=0 -> inter=[v0; u]  => W_A v0 + W_B u + bias
    # T[1*NC + cid] : corr=1 -> inter=[u; v1]  => W_A u + W_B v1 + bias
    T0 = ec @ W_B.T + (W_A @ er[0] + bias)[None, :]
    T1 = ec @ W_A.T + (W_B @ er[1] + bias)[None, :]
    Tbl = np.concatenate([T0, T1], axis=0)

    # device gate order [g, f, i, o]; i,f,o preacts halved so that
    # sigma(a) = 0.5*tanh(a/2)+0.5 becomes 0.5*(tau+1)
    perm = np.concatenate([np.arange(2 * DL, 3 * DL),   # g
                           np.arange(DL, 2 * DL),       # f
                           np.arange(0, DL),            # i
                           np.arange(3 * DL, 4 * DL)])  # o
    beta = np.concatenate([np.full(DL, 1.0),            # g
                           np.full(3 * DL, 0.5)])       # f, i, o
    Tbl = (Tbl[:, perm] * beta[None, :])
    # lstm_fm holds h-tilde' = -12 * 4h = -48h  (the device computes
    # (q-12)*t1 = -12*h~) -> all consumers of lstm_fm divide by -48.
    HS = -48.0
    Whh_eff = (W_hh[perm] * beta[:, None]) / HS
    # gates use Whh @ h~' = Whh @ tq - 12 * Whh @ t1 (tq = t1*q on device)
    WhhT2 = np.stack([-12.0 * Whh_eff.T, Whh_eff.T], axis=1)  # [DL, 2, 4DL]

    bf = np.float16
    return {
        "tbl": np.ascontiguousarray(Tbl).astype(bf),
        "whhT": np.ascontiguousarray(WhhT2).astype(bf),
        "mlpWT": np.ascontiguousarray(inputs["mlp_W"].astype(np.float64).T / HS).astype(bf),
        "mlpb": np.ascontiguousarray(inputs["mlp_b"][None, :]).astype(bf),
        "simW": np.ascontiguousarray(
            np.tile(inputs["sim_W"].reshape(DA, 1), (1, 128))).astype(bf),
        "fcWT": np.ascontiguousarray(
            inputs["fc_W"].astype(np.float64).T / (2.0 * HS)).astype(bf),
        "fcb": np.ascontiguousarray(inputs["fc_b"][None, :] / 2.0).astype(bf),
    }


_CACHE = {}


def kernel(**inputs):
    from concourse.bass_utils import run_bass_kernel_spmd

    has_mlpb = bool(np.any(inputs["mlp_b"] != 0))
    has_fcb = bool(np.any(inputs["fc_b"] != 0))
    key = ("nc", has_mlpb, has_fcb)
    if key not in _CACHE:
        _CACHE[key] = build_kernel(has_mlpb=has_mlpb, has_fcb=has_fcb)
    nc = _CACHE[key]

    shared = _prepare(inputs)
    tbl = shared.pop("tbl")
    cseq = inputs["concept_seq"].astype(np.int64)
    rseq = inputs["correct_seq"].astype(np.int64)
    idx = rseq * NC + cseq                                  # [B, T]
    xg_all = tbl[idx]                                       # [B, T, 8*128]
    # -> [B, nblk, tb, j, p] -> per core [nblk, p, j, b, tb]
    xg_all = xg_all.reshape(B, T // 128, 128, 8, 128)

    in_maps = []
    for i in range(N_CORES):
        m = dict(shared)
        xgc = xg_all[i * BC:(i + 1) * BC]                   # [BC, blk, tb, j, p]
        m["xg"] = np.ascontiguousarray(xgc.transpose(1, 4, 3, 0, 2))
        in_maps.append(m)

    res = run_bass_kernel_spmd(nc, in_maps, list(range(N_CORES)))
    out = np.concatenate(
        [np.asarray(res.results[i]["y"]).astype(np.float32).reshape(BC, T, NC)
         for i in range(N_CORES)], axis=0)
    # odd sequences' last-block rows were emitted as raw logits (their
    # device epilogue was a DVE copy, not a tanh) -- finish them here
    raw = out[:, T - BLK:, :]
    sel = (np.arange(B) % BC) >= 5
    raw[sel] = np.tanh(raw[sel])
    return out * 0.5 + 0.5


# revision 55
# speedup vs baseline: 1.0006x; 1.0003x over previous
"""ATKT (LSTM + degenerate causal attention + FC) Trainium2 kernel, v3.

Full inputs in, full outputs out. Shards batch (64) across 8 NeuronCores
(8 sequences per core), runs a Bass/Tile kernel per core, reassembles.

The Tile scheduler's cost model charges ~100ns (SEM_DELAY) per data-
dependency edge, so the LSTM recurrence (512 serial steps) is bound by
the number of serial edges per step, not engine throughput. v3 cuts the
baseline's 8-edge step to 6 edges:

  Whh-matmuls -> tanh(all 8 gate chunks, one Act) -> STT1 -> STT2
      -> {t1 = (tau_o+1)*s', q = s'^2 on Pool} -> tq = t1*q (Pool)

and feeds the PE *two* rhs streams instead of h:
  gates = xg + Whh @ h~' = xg - 12*Whh @ t1 + Whh @ (t1*q)
(the cubic-tanh h~' = (q-12)*t1 = -12*h~ is materialized off the
critical cycle for attention/FC only; the -1/12 scale is folded into
all weight consumers on the host).

Other restructurings:
 - xg (the interaction-embedding gate table) is gathered AND transposed
   on the host, so phase 1 is four plain DMAs instead of 32 indirect
   gathers + 32 DMA transposes on engine queues.
 - Attention (64-step granularity) and FC (128-token chunks) are
   emitted as thunks and dripped a few per step into the engine-idle
   windows of the recurrence loop; the last sub-block's pipeline is
   ordered per-sequence (scans then that sequence's FC) to minimize the
   tail.
 - The final sigmoid is computed as tanh on-device (table shared with
   the recurrence) and 0.5*y+0.5 on the host; y is fp16.
"""
import os
import sys
from collections import deque

sys.path.insert(0, "/opt/trn_rl_repo")

import numpy as np
import ml_dtypes

B, T = 64, 512
DC = DR = DL = DA = 256
NC = 1024
N_CORES = 8
BC = B // N_CORES          # sequences per core
TOK = BC * T               # tokens per core (4096)
BLK = 128                  # recurrence block size for phase-3 streaming
NBLK = T // BLK

# Custom DVE ops and gpsimd-STT both fail this container's walrus codegen
# ("ISA wrong length" / "engine check failed") -- stock ops only.
USE_CUSTOM_DVE = False
STT_ON_POOL = False

# ----------------------------------------------------------------------------
# Walrus workaround: this container's neuronxcc rejects >1 sync wait per
# instruction ("Too many sync wait commands"). Split multi-wait instructions
# into single-wait NoOps on the same engine.
# ----------------------------------------------------------------------------


def _apply_tile_patches():
    import bass_rust
    import concourse.tile as tile
    from concourse import mybir

    if getattr(tile.TileContext, "_waitsplit_patched", False):
        return

    _orig_lower = tile.TileContext._lower_ordered_insts

    def _split_waits_in_list(uid, insts, counter):
        new_list = []
        for inst in insts:
            si = inst.sync_info
            if si is not None and len(si.on_wait) > 1:
                waits = list(si.on_wait)
                for w in waits[:-1]:
                    counter[0] += 1
                    nop = mybir.InstNoOp(
                        name=f"waitsplit_{uid}_{counter[0]}",
                        engine=inst.engine,
                        sync_info=bass_rust.SyncInfo(on_wait=[w], on_update=[]),
                        bass_nofuse=True,
                    )
                    new_list.append(nop)
                inst.sync_info = bass_rust.SyncInfo(
                    on_wait=[waits[-1]], on_update=list(si.on_update))
            new_list.append(inst)
        return new_list

    def _patched_lower(self, ordered):
        counter = [0]
        for bb_name in list(ordered.keys()):
            ordered[bb_name] = _split_waits_in_list(self.uid, ordered[bb_name], counter)
        return _orig_lower(self, ordered)

    def _patched_drain_and_barrier(self, tick_clock, wait_clock):
        nc = self.nc
        drain_inst = nc.sync.drain()
        wait_clock.add_sem_waits(
            drain_inst.ins, tile.ScopedClock({None: tick_clock.global_clock}))
        si = drain_inst.ins.sync_info
        if si is not None and len(si.on_wait) > 1:
            waits = list(si.on_wait)
            drain_inst.ins.sync_info = bass_rust.SyncInfo(
                on_wait=waits[:1], on_update=list(si.on_update))
            for w in waits[1:]:
                nop = nc.sync.nop(nofuse=True)
                nop.ins.sync_info = bass_rust.SyncInfo(on_wait=[w], on_update=[])
        nc.all_engine_barrier()
        assert self.sems is not None
        popped = nc._tile_sem_poison_stack.pop()
        assert popped is self._sem_poison
        nc.clear_and_free_semaphores(list(self.sems.allocated().values()))
        nc.all_engine_barrier()

    tile.TileContext._lower_ordered_insts = _patched_lower
    tile.TileContext._drain_and_barrier = _patched_drain_and_barrier
    tile.TileContext._waitsplit_patched = True


# ----------------------------------------------------------------------------
# Custom DVE op: h~ = (tau_o + 1) * s * (1 + C2 * s^2), C2 = -1/12.
# Registered through the documented dve_ops extension point (OPS list);
# the uop program ships in the per-NEFF DVE table, no firmware change.
# ----------------------------------------------------------------------------

_HTILDE = None


def _register_htilde():
    global _HTILDE
    if _HTILDE is not None:
        return _HTILDE
    import concourse.dve_ops as dve_ops
    from concourse.dve_ops import DveOp, OPS, CUSTOM_DVE_SPECS, _SUB_OPCODE_FOR_NAME
    from concourse.dve_spec import Spec, Src0, Src1, C2, One, sq, lower
    from concourse.dve_uop import DveOpSpec, DveVer

    name = "ATKT_HTILDE"
    if name in _SUB_OPCODE_FOR_NAME:
        _HTILDE = next(op for op in OPS if op.name == name)
        return _HTILDE

    body = (Src0 + One) * Src1 * (sq(Src1) * C2 + One)
    spec = Spec(
        body=body,
        reference=lambda in0, in1, s0, s1, imm2: (
            (in0.astype(np.float32) + 1.0)
            * in1.astype(np.float32)
            * (np.square(in1.astype(np.float32)) * imm2 + 1.0)
        ),
    )
    row = max(_SUB_OPCODE_FOR_NAME.values()) + 1
    assert row < 0x20
    _SUB_OPCODE_FOR_NAME[name] = row

    shas = {}
    for ver in ("v3", "v4"):
        uops = lower(spec, ver=ver)
        tmp = DveOpSpec(name=name, opcode=row, uops=uops, rd1_en=True)
        shas[ver] = tmp.sha(ver)
    op = DveOp(name, spec, subdim=False, uops_sha=shas)
    OPS.append(op)
    CUSTOM_DVE_SPECS[name] = spec
    _HTILDE = op
    return op


# ----------------------------------------------------------------------------
# Kernel build
# ----------------------------------------------------------------------------

def build_kernel(t_steps=T, has_mlpb=False, has_fcb=False):
    import concourse.bass as bass
    import concourse.tile as tile
    from concourse import mybir

    _apply_tile_patches()
    htilde_op = _register_htilde() if USE_CUSTOM_DVE else None

    f32 = mybir.dt.float32
    # fp16: |attn_excl| reaches ~320 and fc logits ~43; bf16's 8-bit mantissa
    # costs 4e-2 output error, fp16's 10 bits is fine.
    bf16 = mybir.dt.float16
    i32 = mybir.dt.int32
    AF = mybir.ActivationFunctionType
    OP = mybir.AluOpType

    nc = bass.Bass("TRN2", target_bir_lowering=False, debug=False,
                   num_devices=N_CORES)

    n_tok = BC * t_steps
    n_tc = n_tok // 128            # 128-token chunks
    tc_per_seq = t_steps // 128
    nblk = t_steps // BLK

    # ---- DRAM parameters (per core) ----
    # xg is gathered AND transposed on the host: [blk, p, j, b, t-in-block]
    xg_d = nc.dram_tensor(
        "xg", [t_steps // BLK, 128, 8, BC, BLK], bf16,
        kind="ExternalInput").ap()
    whhT = nc.dram_tensor("whhT", [DL, 2, 4 * DL], bf16, kind="ExternalInput").ap()
    mlpWT = nc.dram_tensor("mlpWT", [DL, DA], bf16, kind="ExternalInput").ap()
    mlpb = nc.dram_tensor("mlpb", [1, DA], bf16, kind="ExternalInput").ap()
    simW = nc.dram_tensor("simW", [DA, 128], bf16, kind="ExternalInput").ap()
    fcWT = nc.dram_tensor("fcWT", [2 * DL, NC], bf16, kind="ExternalInput").ap()
    fcb = nc.dram_tensor("fcb", [1, NC], bf16, kind="ExternalInput").ap()
    yout = nc.dram_tensor("y", [n_tok, NC], bf16, kind="ExternalOutput").ap()

    with tile.TileContext(nc) as tc:
        import contextlib
        with contextlib.ExitStack() as ctx:
            g_pool = ctx.enter_context(tc.tile_pool(name="globals", bufs=1))
            lstm_pool = ctx.enter_context(tc.tile_pool(name="lstm", bufs=1))
            xg_pool = ctx.enter_context(tc.tile_pool(name="xg", bufs=1))
            row_pool = ctx.enter_context(tc.tile_pool(name="rows", bufs=6))
            p3_pool = ctx.enter_context(tc.tile_pool(name="p3", bufs=1))
            p3s_pool = ctx.enter_context(tc.tile_pool(name="p3scratch", bufs=2))
            p3o_pool = ctx.enter_context(tc.tile_pool(name="p3out", bufs=2))
            ps_g = ctx.enter_context(
                tc.tile_pool(name="ps_gates", bufs=2, space="PSUM"))
            ps_att = ctx.enter_context(
                tc.tile_pool(name="ps_att", bufs=1, space="PSUM"))
            ps_out = ctx.enter_context(
                tc.tile_pool(name="ps_out", bufs=4, space="PSUM"))

            # ---- persistent small tiles ----
            ones_f = g_pool.tile([128, 512], f32)
            nc.vector.memset(ones_f, 1.0)
            ones_b = g_pool.tile([128, 512], bf16)
            nc.vector.memset(ones_b, 1.0)
            ident_b = g_pool.tile([128, 128], bf16)
            nc.vector.memset(ident_b, 1.0)
            nc.gpsimd.affine_select(
                out=ident_b, in_=ident_b, pattern=[[-1, 128]],
                compare_op=OP.is_equal, fill=0.0, base=0, channel_multiplier=1)

            whh_sb = g_pool.tile([128, 2, 2, 4 * DL], bf16)
            whh_r = whhT.rearrange("(k p) s g -> p k s g", p=128)
            # split across two queues so neither DMA alone gates step 0
            nc.gpsimd.dma_start(out=whh_sb[:, :, 0, :], in_=whh_r[:, :, 0, :])
            nc.scalar.dma_start(out=whh_sb[:, :, 1, :], in_=whh_r[:, :, 1, :])

            # charge the activation-table load before step 0's tanh
            warm = g_pool.tile([128, 1], f32, name="warm")
            nc.scalar.activation(out=warm, in_=ones_f[:, 0:1], func=AF.Tanh)

            # X: recurrence state + taus, fp16:
            # slots [0:2]=s  [2:4]=tau_g  [4:6]=tau_f  [6:8]=tau_i  [8:10]=tau_o
            X = g_pool.tile([128, 10, BC], bf16)
            nc.vector.memset(X[:, 0:2, :], 0.0)
            W2 = g_pool.tile([128, 4, BC], bf16)
            op1_t = g_pool.tile([128, 2, BC], bf16)
            q_t = g_pool.tile([128, 2, BC], bf16)
            # ping-pong: step t writes slot t%2, the matmuls of step t+1 read it
            t1_t = g_pool.tile([128, 2, 2, BC], bf16)
            nc.vector.memset(t1_t.rearrange("p s k b -> p (s k b)"), 0.0)
            tq_t = g_pool.tile([128, 2, 2, BC], bf16)
            nc.vector.memset(tq_t.rearrange("p s k b -> p (s k b)"), 0.0)

            # lstm_out (h-tilde = 4h) feature-major: [p, k(2 H-chunks), b, t]
            lstm_fm = lstm_pool.tile([128, 2, BC, t_steps], bf16)
            # attention persistents
            attn_t = g_pool.tile([128, 2, BC, t_steps], bf16, tag="attn")
            excl_t = g_pool.tile([128, 2, BC, t_steps], bf16, tag="excl")
            nc.vector.memset(
                excl_t[:, :, :, 0:1].rearrange("p k b o -> p (k b o)"), 0.0)

            # ---- phase-3 weights (off the SP queue: transposes need it) ----
            mlp_sb = p3_pool.tile([128, 2, DA], bf16)
            nc.gpsimd.dma_start(out=mlp_sb,
                                in_=mlpWT.rearrange("(k p) a -> p k a", p=128))
            simrep_sb = p3_pool.tile([128, 2, 128], bf16)
            nc.gpsimd.dma_start(out=simrep_sb,
                                in_=simW.rearrange("(k p) o -> p k o", p=128))
            if has_mlpb:
                mlpb_sb = p3_pool.tile([1, DA], bf16)
                nc.gpsimd.dma_start(out=mlpb_sb, in_=mlpb)
            fc_sb = p3_pool.tile([128, 4, NC], bf16)
            nc.gpsimd.dma_start(out=fc_sb,
                                in_=fcWT.rearrange("(k p) c -> p k c", p=128))
            if has_fcb:
                fcb_sb = p3_pool.tile([1, NC], bf16)
                nc.gpsimd.dma_start(out=fcb_sb, in_=fcb)



            # xg per block: [p, j(8 gate chunks), b, t-in-block]
            xg_tiles = {}

            def xg_tile(blk):
                if blk not in xg_tiles:
                    xg_tiles[blk] = xg_pool.tile(
                        [128, 8, BC, BLK], bf16, tag=f"xg{blk % 2}",
                        name=f"xgb{blk}")
                return xg_tiles[blk]

            xg_loaded = set()

            def emit_xg_dma(blk, parts=2):
                if blk in xg_loaded or blk >= nblk:
                    return
                xg_loaded.add(blk)
                step = BLK // parts
                xt = xg_tile(blk)
                for pi in range(parts):
                    nc.sync.dma_start(
                        out=xt[:, :, :, pi * step:(pi + 1) * step],
                        in_=xg_d[blk, :, :, :, pi * step:(pi + 1) * step])

            emit_xg_dma(0, parts=8)
            emit_xg_dma(1)

            # ============ Phase 3 emission (as thunks) ============
            # Attention pipeline at 64-step granularity; FC per 128-token
            # chunk.  Thunks are dripped a few per step into the engine-idle
            # windows of the recurrence loop.
            att_pool = p3s_pool  # scratch pools rotate via bufs=2
            SUB = 64
            cwr_tiles = {}
            cum_tiles = {}

            def att_thunks(sb):
                """Attention pipeline for t-range [sb*SUB, (sb+1)*SUB)."""
                t0 = sb * SUB
                th = []
                att_n = att_pool.tile([128, 2, BC, SUB], bf16, tag="attn_s",
                                      name=f"attn{sb}")
                wrep = att_pool.tile([128, BC, SUB], bf16, tag="wrep",
                                     name=f"wrep{sb}")
                cwr = att_pool.tile([128, BC, SUB], f32, tag="cwr",
                                    name=f"cwr{sb}")
                rwr = att_pool.tile([128, BC, SUB], f32, tag="rwr",
                                    name=f"rwr{sb}")
                wh = att_pool.tile([128, 2, BC, SUB], bf16, tag="wh",
                                   name=f"wh{sb}")
                cum = att_pool.tile([128, 2, BC, SUB], f32, tag="cum",
                                    name=f"cum{sb}")
                cwr_prev = cwr_tiles.get(sb - 1)
                cum_prev = cum_tiles.get(sb - 1)
                cwr_tiles[sb] = cwr
                cum_tiles[sb] = cum

                # --- att = tanh(mlp_W @ h) ---
                # two shared PSUM tags; mm-group then act-group ordering so
                # dripped acts are ready the moment the Act engine is free.
                def att_mm_group(m, bh, aps):
                    out = []
                    for bi in range(4):
                        bq = bh * 4 + bi
                        for k2 in range(2):
                            def mm(m=m, bi=bi, bq=bq, k2=k2, aps=aps):
                                nc.tensor.matmul(
                                    out=aps[:, bi, :],
                                    lhsT=mlp_sb[:, k2, 128 * m:128 * (m + 1)],
                                    rhs=lstm_fm[:, k2, bq, t0:t0 + SUB],
                                    start=(k2 == 0),
                                    stop=(k2 == 1 and not has_mlpb))
                            out.append(mm)
                        if has_mlpb:
                            def mmb(m=m, bi=bi, aps=aps):
                                nc.tensor.matmul(
                                    out=aps[:, bi, :],
                                    lhsT=mlpb_sb[:, 128 * m:128 * (m + 1)],
                                    rhs=ones_b[0:1, :SUB],
                                    start=False, stop=True)
                            out.append(mmb)
                    return out

                def att_act(m, bh, aps):
                    def act(m=m, bh=bh, aps=aps):
                        nc.scalar.activation(
                            out=att_n[:, m, bh * 4:bh * 4 + 4, :].rearrange(
                                "p b t -> p (b t)"),
                            in_=aps.rearrange("p b t -> p (b t)"),
                            func=AF.Tanh)
                    return act

                # bh0's full chain (att m0, m1 -> score -> exp) first so
                # the bh0 scans can start while bh1's atts still run.
                for bh in range(2):
                    aps0 = ps_att.tile([128, 4, SUB], f32, tag="ab0",
                                       name=f"aps{sb}_0{bh}")
                    th.extend(att_mm_group(0, bh, aps0))
                    aps1 = ps_att.tile([128, 4, SUB], f32, tag="ab1",
                                       name=f"aps{sb}_1{bh}")
                    th.extend(att_mm_group(1, bh, aps1))
                    th.append(att_act(0, bh, aps0))
                    th.append(att_act(1, bh, aps1))
                    bps = ps_att.tile([128, 4, SUB], f32, tag="ab0",
                                      name=f"bps{sb}_{bh}")
                    for bi in range(4):
                        bq = bh * 4 + bi
                        for m in range(2):
                            def mm(bi=bi, bq=bq, m=m, bps=bps):
                                nc.tensor.matmul(
                                    out=bps[:, bi, :],
                                    lhsT=simrep_sb[:, m, :],
                                    rhs=att_n[:, m, bq, :],
                                    start=(m == 0), stop=(m == 1))
                            th.append(mm)
                    def act(bh=bh, bps=bps):
                        nc.scalar.activation(
                            out=wrep[:, bh * 4:bh * 4 + 4, :].rearrange(
                                "p b t -> p (b t)"),
                            in_=bps.rearrange("p b t -> p (b t)"),
                            func=AF.Exp)
                    th.append(act)

                # --- per-sequence scan groups: cwr, recip, wh, cum,
                # attn, excl for one b.  Emitted per-b so the tail can
                # interleave each sequence's FC right after its scans. ---
                n_out = SUB if t0 + SUB < t_steps else SUB - 1
                per_b = []
                for bq in range(BC):
                    g = []
                    def scn_w(bq=bq, sb=sb):
                        init = (0.0 if sb == 0
                                else cwr_prev[:, bq, SUB - 1:SUB])
                        nc.vector.tensor_tensor_scan(
                            out=cwr[:, bq, :], data0=ones_f[:, :SUB],
                            data1=wrep[:, bq, :],
                            initial=init, op0=OP.mult, op1=OP.add)
                    g.append(scn_w)
                    def rcp(bq=bq):
                        nc.vector.reciprocal(
                            out=rwr[:, bq, :], in_=cwr[:, bq, :])
                    g.append(rcp)
                    for k2 in range(2):
                        def mul_wh(k2=k2, bq=bq):
                            nc.gpsimd.tensor_mul(
                                wh[:, k2, bq, :],
                                wrep[:, bq, :],
                                lstm_fm[:, k2, bq, t0:t0 + SUB])
                        g.append(mul_wh)
                    for k2 in range(2):
                        def scn_c(k2=k2, bq=bq, sb=sb):
                            init = (0.0 if sb == 0
                                    else cum_prev[:, k2, bq, SUB - 1:SUB])
                            nc.vector.tensor_tensor_scan(
                                out=cum[:, k2, bq, :],
                                data0=ones_f[:, :SUB],
                                data1=wh[:, k2, bq, :],
                                initial=init, op0=OP.mult, op1=OP.add)
                        g.append(scn_c)
                    for k2 in range(2):
                        def mul_a(k2=k2, bq=bq):
                            nc.gpsimd.tensor_mul(
                                attn_t[:, k2, bq, t0:t0 + SUB],
                                cum[:, k2, bq, :],
                                rwr[:, bq, :])
                        g.append(mul_a)
                    for k2 in range(2):
                        def scn_e(k2=k2, bq=bq, n_out=n_out, t0=t0):
                            nc.vector.tensor_tensor_scan(
                                out=excl_t[:, k2, bq, t0 + 1:t0 + 1 + n_out],
                                data0=ones_f[:, :n_out],
                                data1=attn_t[:, k2, bq, t0:t0 + n_out],
                                initial=excl_t[:, k2, bq, t0:t0 + 1],
                                op0=OP.mult, op1=OP.add)
                        g.append(scn_e)
                    per_b.append(g)
                return th, per_b

            def fc_chunk(blk, bq, t0=None, tlen=BLK, epilogue="act"):
                """FC for one token chunk (bq, [t0, t0+tlen)).  epilogue
                "act" = tanh on Act; "dve" = raw-logit copy on DVE (the host
                applies the tanh for those rows)."""
                if t0 is None:
                    t0 = blk * BLK
                th = []
                ysbs[(bq, t0)] = p3o_pool.tile([tlen, NC], bf16, tag="ysb",
                                               name=f"ysb{t0}_{bq}")
                for half in range(2):
                    ops = ps_out.tile([tlen, 512], f32, tag="ops",
                                      name=f"ops{t0}_{bq}_{half}")
                    srcs = [(excl_t, 0), (excl_t, 1),
                            (lstm_fm, 0), (lstm_fm, 1)]
                    for k4, (src, k2) in enumerate(srcs):
                        def mm(bq=bq, half=half, k4=k4, src=src, k2=k2,
                               ops=ops, t0=t0):
                            nc.tensor.matmul(
                                out=ops,
                                lhsT=src[:, k2, bq, t0:t0 + tlen],
                                rhs=fc_sb[:, k4,
                                          512 * half:512 * (half + 1)],
                                start=(k4 == 0),
                                stop=(k4 == 3 and not has_fcb))
                        th.append(mm)
                    if has_fcb:
                        def mmb(half=half, ops=ops):
                            nc.tensor.matmul(
                                out=ops, lhsT=ones_b[0:1, 0:tlen],
                                rhs=fcb_sb[:, 512 * half:512 * (half + 1)],
                                start=False, stop=True)
                        th.append(mmb)
                    if epilogue == "act":
                        def act(half=half, ops=ops, bq=bq, t0=t0):
                            nc.scalar.activation(
                                out=ysbs[(bq, t0)][:, 512 * half:
                                                   512 * (half + 1)],
                                in_=ops, func=AF.Tanh)
                        th.append(act)
                    else:
                        def cpy(half=half, ops=ops, bq=bq, t0=t0):
                            nc.vector.tensor_copy(
                                out=ysbs[(bq, t0)][:, 512 * half:
                                                   512 * (half + 1)],
                                in_=ops)
                        th.append(cpy)
                def dma(bq=bq, t0=t0):
                    r0 = bq * t_steps + t0
                    nc.sync.dma_start(out=yout[r0:r0 + tlen, :],
                                      in_=ysbs[(bq, t0)])
                th.append(dma)
                return th

            ysbs = {}
            drip = deque()

            # ================= Phase 2: LSTM recurrence ================
            # gate-chunk order in gps: [g, f, i, o] (host perm).  Cell math:
            #   s'  = 0.5*(tau_f+1)*s + (tau_i+1)*tau_g          (s = 2c)
            #   h~  = (tau_o+1) * s' * (1 - s'^2/12)   (~= 4h, cubic tanh)
            for t in range(t_steps):
                blk = t // BLK
                pp, ppn = (t + 1) % 2, t % 2
                gps = ps_g.tile([128, 8, BC], f32, tag="gps")
                # xg preload (off critical path: no h dependency)
                nc.tensor.matmul(
                    out=gps.rearrange("p j b -> p (j b)"),
                    lhsT=ident_b,
                    rhs=xg_tile(blk)[:, :, :, t - blk * BLK].rearrange(
                        "p j b -> p (j b)"),
                    start=True, stop=False)
                # gates += -12*Whh @ t1  (t1 lands ~113ns before tq)
                for j in range(8):
                    for k2 in range(2):
                        nc.tensor.matmul(
                            out=gps[:, j, :],
                            lhsT=whh_sb[:, k2, 0, 128 * j:128 * (j + 1)],
                            rhs=t1_t[:, pp, k2, :],
                            start=False, stop=False)
                # gates += Whh @ (t1*q)   [== Whh @ h-tilde']
                for j in range(8):
                    for k2 in range(2):
                        nc.tensor.matmul(
                            out=gps[:, j, :],
                            lhsT=whh_sb[:, k2, 1, 128 * j:128 * (j + 1)],
                            rhs=tq_t[:, pp, k2, :],
                            start=False, stop=(j == 7 and k2 == 1))
                # one Act: all 8 gate chunks -> taus
                nc.scalar.activation(
                    out=X[:, 2:10, :].rearrange("p j b -> p (j b)"),
                    in_=gps.rearrange("p j b -> p (j b)"),
                    func=AF.Tanh)
                nc.gpsimd.tensor_scalar(
                    out=op1_t.rearrange("p j b -> p (j b)"),
                    in0=X[:, 8:10, :].rearrange("p j b -> p (j b)"),
                    scalar1=1.0, scalar2=1.0, op0=OP.add, op1=OP.mult)
                # STT1: [w1; v] = (X[4:8]+1) * [s; tau_g]
                nc.vector.scalar_tensor_tensor(
                    out=W2.rearrange("p j b -> p (j b)"),
                    in0=X[:, 4:8, :].rearrange("p j b -> p (j b)"),
                    scalar=1.0,
                    in1=X[:, 0:4, :].rearrange("p j b -> p (j b)"),
                    op0=OP.add, op1=OP.mult)
                # STT2: s' = 0.5*w1 + v
                nc.vector.scalar_tensor_tensor(
                    out=X[:, 0:2, :].rearrange("p j b -> p (j b)"),
                    in0=W2[:, 0:2, :].rearrange("p j b -> p (j b)"),
                    scalar=0.5,
                    in1=W2[:, 2:4, :].rearrange("p j b -> p (j b)"),
                    op0=OP.mult, op1=OP.add)
                sfl = X[:, 0:2, :].rearrange("p j b -> p (j b)")
                hout = lstm_fm[:, :, :, t].rearrange("p k b -> p (k b)")
                # tier 5 on Pool: t1 = (tau_o+1)*s' first (tq needs it), then
                # q = s'^2
                qfl = q_t.rearrange("p j b -> p (j b)")
                t1fl = t1_t[:, ppn].rearrange("p k b -> p (k b)")
                tqfl = tq_t[:, ppn].rearrange("p k b -> p (k b)")
                nc.gpsimd.tensor_mul(
                    t1fl, op1_t.rearrange("p j b -> p (j b)"), sfl)
                nc.gpsimd.tensor_mul(qfl, sfl, sfl)
                # tier 6 on Pool (critical): tq = t1*q; the next step's gates
                # use  Whh @ h~' = Whh @ tq - 12*Whh @ t1.
                nc.gpsimd.tensor_mul(tqfl, t1fl, qfl)
                # off the critical cycle: h~' = (q-12)*t1 for attention/FC
                nc.vector.scalar_tensor_tensor(
                    out=hout, in0=qfl, scalar=-12.0, in1=t1fl,
                    op0=OP.add, op1=OP.mult)

                # ---- prefetch next xg block (resource-gated by xg slot) ----
                if t % BLK == 0 and t > 0:
                    emit_xg_dma(t // BLK + 1)
                # ---- drip: phase-3 thunks ----
                if t % 64 == 63 and t < t_steps - 1:
                    sb = t // 64
                    pre, per_b = att_thunks(sb)
                    drip.extend(pre)
                    for bq in range(BC):
                        drip.extend(per_b[bq])
                if t % BLK == BLK - 1 and t < t_steps - 1 and blk < nblk - 1:
                    for bq in range(BC):
                        drip.extend(fc_chunk(blk, bq))
                if drip:
                    quota = max(3, (len(drip) + 55) // 56)
                    for _ in range(min(quota, len(drip))):
                        drip.popleft()()

            # tail: last sub-block's attention per-b, each followed by the
            # sequence's FC chunk; tiny warmup matmuls keep the PE p-state hot.
            while drip:
                drip.popleft()()
            warm_ps = ps_att.tile([128, 4, SUB], f32, tag="ab0", name="warmps")

            def pe_warm():
                nc.tensor.matmul(out=warm_ps[:, 0, 0:1], lhsT=ident_b,
                                 rhs=ones_b[:, 0:1], start=True, stop=True)

            last_sb = t_steps // 64 - 1
            pre, per_b = att_thunks(last_sb)
            for f in pre:
                f()
            pe_warm()
            for bq in range(BC):
                for f in per_b[bq]:
                    f()
                pe_warm()
                for f in fc_chunk(nblk - 1, bq,
                                  epilogue="act"):
                    f()

    return nc


# ----------------------------------------------------------------------------
# Host-side weight preparation
# ----------------------------------------------------------------------------

def _prepare(inputs):
    W_ih = inputs["W_ih"].astype(np.float64)
    W_hh = inputs["W_hh"].astype(np.float64)
    b_ih = inputs["b_ih"].astype(np.float64)
    b_hh = inputs["b_hh"].astype(np.float64)
    ec = inputs["embed_concept"].astype(np.float64)
    er = inputs["embed_correct"].astype(np.float64)

    W_A = W_ih[:, :DC]
    W_B = W_ih[:, DC:]
    bias = b_ih + b_hh
    # T[0*NC + cid] : corr=0 -> inter=[v0; u]  => W_A v0 + W_B u + bias
    # T[1*NC + cid] : corr=1 -> inter=[u; v1]  => W_A u + W_B v1 + bias
    T0 = ec @ W_B.T + (W_A @ er[0] + bias)[None, :]
    T1 = ec @ W_A.T + (W_B @ er[1] + bias)[None, :]
    Tbl = np.concatenate([T0, T1], axis=0)

    # device gate order [g, f, i, o]; i,f,o preacts halved so that
    # sigma(a) = 0.5*tanh(a/2)+0.5 becomes 0.5*(tau+1)
    perm = np.concatenate([np.arange(2 * DL, 3 * DL),   # g
                           np.arange(DL, 2 * DL),       # f
                           np.arange(0, DL),            # i
                           np.arange(3 * DL, 4 * DL)])  # o
    beta = np.concatenate([np.full(DL, 1.0),            # g
                           np.full(3 * DL, 0.5)])       # f, i, o
    Tbl = (Tbl[:, perm] * beta[None, :])
    # lstm_fm holds h-tilde' = -12 * 4h = -48h  (the device computes
    # (q-12)*t1 = -12*h~) -> all consumers of lstm_fm divide by -48.
    HS = -48.0
    Whh_eff = (W_hh[perm] * beta[:, None]) / HS
    # gates use Whh @ h~' = Whh @ tq - 12 * Whh @ t1 (tq = t1*q on device)
    WhhT2 = np.stack([-12.0 * Whh_eff.T, Whh_eff.T], axis=1)  # [DL, 2, 4DL]

    bf = np.float16
    return {
        "tbl": np.ascontiguousarray(Tbl).astype(bf),
        "whhT": np.ascontiguousarray(WhhT2).astype(bf),
        "mlpWT": np.ascontiguousarray(inputs["mlp_W"].astype(np.float64).T / HS).astype(bf),
        "mlpb": np.ascontiguousarray(inputs["mlp_b"][None, :]).astype(bf),
        "simW": np.ascontiguousarray(
            np.tile(inputs["sim_W"].reshape(DA, 1), (1, 128))).astype(bf),
        "fcWT": np.ascontiguousarray(
            inputs["fc_W"].astype(np.float64).T / (2.0 * HS)).astype(bf),
        "fcb": np.ascontiguousarray(inputs["fc_b"][None, :] / 2.0).astype(bf),
    }


_CACHE = {}


def kernel(**inputs):
    from concourse.bass_utils import run_bass_kernel_spmd

    has_mlpb = bool(np.any(inputs["mlp_b"] != 0))
    has_fcb = bool(np.any(inputs["fc_b"] != 0))
    key = ("nc", has_mlpb, has_fcb)
    if key not in _CACHE:
        _CACHE[key] = build_kernel(has_mlpb=has_mlpb, has_fcb=has_fcb)
    nc = _CACHE[key]

    shared = _prepare(inputs)
    tbl = shared.pop("tbl")
    cseq = inputs["concept_seq"].astype(np.int64)
    rseq = inputs["correct_seq"].astype(np.int64)
    idx = rseq * NC + cseq                                  # [B, T]
    xg_all = tbl[idx]                                       # [B, T, 8*128]
    # -> [B, nblk, tb, j, p] -> per core [nblk, p, j, b, tb]
    xg_all = xg_all.reshape(B, T // 128, 128, 8, 128)

    in_maps = []
    for i in range(N_CORES):
        m = dict(shared)
        xgc = xg_all[i * BC:(i + 1) * BC]                   # [BC, blk, tb, j, p]
        m["xg"] = np.ascontiguousarray(xgc.transpose(1, 4, 3, 0, 2))
        in_maps.append(m)

    res = run_bass_kernel_spmd(nc, in_maps, list(range(N_CORES)))
    out = np.concatenate(
        [np.asarray(res.results[i]["y"]).astype(np.float32).reshape(BC, T, NC)
         for i in range(N_CORES)], axis=0)
    return out * 0.5 + 0.5
